# revision 24
# baseline (speedup 1.0000x reference)
"""BasicTransformerBlock (self-attn + cross-attn + GEGLU FF) on 8 TRN2 cores.

Sharding: sequence-parallel, no collectives. B=4 batches x 2 sequence-halves
= 8 shards; each core computes 512 query rows end-to-end, duplicating only
the (cheap) K/V projections for its batch. The host rolls each batch's
hidden_states so a core's query rows are always rows 0..511 — the kernel is
uniform SPMD.

v2: fp8 (e4m3, TRN float8e4) everywhere on the matmul path with DoubleRow
perf mode (contraction-256 per instruction):
  - weights host-quantized to fp8 at x64 scale; activations quantized on
    device (LN outputs std 1 -> direct; attn out via 1/256 staging scale)
  - Q/K projections use host-permuted weight columns so per-head dh=64 is
    laid out as [32 partitions x 2 pair-slots], letting the scores matmul
    run DoubleRow with 4 heads packed per 128 partitions (row tiling)
  - PV accumulates 4 heads per PSUM bank; softmax denominator via fused
    65th ones-column of V; batched normalize per s-chunk
  - residuals stay f32; out-proj/FF biases folded in as rank-1 bf16 matmuls
  - engine balance: exp/gelu/K-epilogues on ACT, LN-normalize + FF1-mult on
    GpSimd (Pool), everything PSUM-touching on DVE
"""

import contextlib
import os

import numpy as np

_KSTOP = int(os.environ.get("KSTOP", "99"))

import concourse.mybir as mybir
import concourse.tile as tile
from concourse import bacc
from concourse.bass_utils import run_bass_kernel_spmd
from concourse.masks import make_identity

P = 128
B, S, T, D, H, DH = 4, 1024, 1024, 1024, 16, 64
FF = 4 * D
SQ = 512                 # query rows per core
SCALE = DH ** -0.5
EPS = 1e-12
NCORES = 8

WS = 64.0                # weight fp8 scale
MS = 8.0                 # ff1 hidden (mT) fp8 scale
PVS = 256.0              # PV psum -> fp8 staging scale

f32 = mybir.dt.float32
f32r = mybir.dt.float32r
bf16 = mybir.dt.bfloat16
f8 = mybir.dt.float8e4
AF = mybir.ActivationFunctionType
ALU = mybir.AluOpType
DR = mybir.MatmulPerfMode.DoubleRow

DSUB = D // P            # 8
TSUB = T // P            # 8
SSUB = S // P            # 8
QSUB = SQ // P           # 4
FSUB = FF // P           # 32
KP = DSUB // 2           # 4 contraction k-pairs per D-deep matmul


# --------------------------------------------------------------------------
# device-program helpers
# --------------------------------------------------------------------------

def _ln_tile(nc, sb_small, x_ap, xn8_ap, eps_ap):
    """xn8 = (x - mean) * rsqrt(var + eps), written fp8 on Pool."""
    stats = sb_small.tile([P, D // 512, 6], f32, tag="ln_stats")
    for c in range(D // 512):
        nc.vector.bn_stats(stats[:, c], x_ap[:, c * 512:(c + 1) * 512])
    mv = sb_small.tile([P, 2], f32, tag="ln_mv")
    nc.vector.bn_aggr(mv, stats)
    std = sb_small.tile([P, 1], f32, tag="ln_std")
    nc.scalar.activation(std, mv[:, 1:2], AF.Sqrt, bias=eps_ap)
    rstd = sb_small.tile([P, 1], f32, tag="ln_rstd")
    nc.vector.reciprocal(rstd, std)
    nc.gpsimd.tensor_scalar(xn8_ap, x_ap, mv[:, 0:1], rstd,
                            ALU.subtract, ALU.mult)


def _transpose4(nc, ps_tr, ident8, src_f8, dst4, eng):
    """Transpose 4 adjacent [128,128] fp8 blocks; one merged copy to dst4.

    src_f8: [P, 512] fp8 AP (4 d-blocks); dst4: [P, 4, 128] fp8 AP.
    eng: 'act' or 'dve' for the PSUM->SBUF copy.
    """
    tp = ps_tr.tile([P, 4, P], bf16, tag="tr_ps")
    for i in range(4):
        nc.tensor.transpose(tp[:, i], src_f8[:, i * P:(i + 1) * P], ident8)
    if eng == "act":
        nc.scalar.copy(dst4, tp)
    else:
        nc.vector.tensor_copy(dst4, tp)


def _load_w8(nc, wpool, w_dr):
    """Stream a [D, O] fp8 weight as KP tiles [128, 2, O] (k = ks*128+p)."""
    O = w_dr.shape[1]
    parts = []
    for q in range(KP):
        wt = wpool.tile([P, 2, O], f8, tag="w")
        nc.sync.dma_start(
            wt, w_dr.rearrange("(ks p) o -> p ks o", p=P)[:, 2 * q:2 * q + 2])
        parts.append(wt)
    return parts


def _proj_qk(nc, wpool, ps_pool, w_dr, rhsT, outT8, bias_s, ncols, eng):
    """outT8[:, dsb//2, dsb%2, :] = (W.T @ xn)[d-chunk dsb] / WS + bias.

    rhsT: [P, DSUB, ncols] fp8; outT8: [P, 4, 2, ncols] fp8 (scores layout);
    bias_s: [P, DSUB] (host-permuted) or None. eng: epilogue engine.
    """
    halves = _load_w8(nc, wpool, w_dr)
    nhalf = ncols // 512
    for dsb in range(DSUB):
        for ch in range(nhalf):
            ps = ps_pool.tile([P, 512], f32, tag="proj")
            for q in range(KP):
                nc.tensor.matmul(
                    ps,
                    halves[q][:, :, dsb * P:(dsb + 1) * P],
                    rhsT[:, 2 * q:2 * q + 2, ch * 512:(ch + 1) * 512],
                    start=(q == 0), stop=(q == KP - 1), perf_mode=DR,
                )
            dst = outT8[:, dsb // 2, dsb % 2, ch * 512:(ch + 1) * 512]
            if eng == "act":
                b = bias_s[:, dsb:dsb + 1] if bias_s is not None else 0.0
                nc.scalar.activation(dst, ps, AF.Identity, bias=b,
                                     scale=1.0 / WS)
            else:
                b = bias_s[:, dsb:dsb + 1] if bias_s is not None else 0.0
                nc.vector.tensor_scalar(dst, ps, 1.0 / WS, b,
                                        ALU.mult, ALU.add)


def _proj_v(nc, wpool, ps_pool, w_dr, lhsT8, v8, vb_b, maskd_s):
    """V[t, dv] natural fp8, per head, 65th column = ones (or mask).

    v8: [P, TSUB, H, 65] fp8. If maskd_s given ([P,TSUB] = mask/WS), V rows
    are scaled by mask (exp(score)*m path); else vb_b [P, D] bias is added.
    """
    halves = _load_w8(nc, wpool, w_dr)
    for ts in range(TSUB):
        for dh in range(2):
            ps = ps_pool.tile([P, 512], f32, tag="proj")
            for q in range(KP):
                nc.tensor.matmul(
                    ps,
                    lhsT8[:, 2 * q:2 * q + 2, ts * P:(ts + 1) * P],
                    halves[q][:, :, dh * 512:(dh + 1) * 512],
                    start=(q == 0), stop=(q == KP - 1), perf_mode=DR,
                )
            dst = v8[:, ts, dh * 8:(dh + 1) * 8, 0:64]
            src = ps.rearrange("p (h w) -> p h w", h=8)
            if maskd_s is None:
                nc.vector.scalar_tensor_tensor(
                    dst, src, 1.0 / WS,
                    vb_b[:, dh * 512:(dh + 1) * 512].rearrange(
                        "p (h w) -> p h w", h=8),
                    ALU.mult, ALU.add)
            else:
                nc.vector.tensor_scalar(dst, src, maskd_s[:, ts:ts + 1], None,
                                        ALU.mult)
    if maskd_s is None:
        nc.vector.memset(v8[:, :, :, 64:65], 1.0 / WS)
    else:
        for ts in range(TSUB):
            nc.vector.tensor_copy(
                v8[:, ts, :, 64],
                maskd_s[:, ts:ts + 1].to_broadcast((P, H)))


def _attention(nc, ET_pool, ps_sc, ps_pv, sb_small, qT8, kT8, v8, attn_un):
    """scores^T (fp8 DoubleRow, 4 heads/row-tile) -> exp -> PV -> attn_un.

    attn_un: [P, QSUB, H, 65] fp8 = unnormalized PV / PVS. Column 64 holds
    sum(E)/WS (the ones-column of V is 1/WS), so normalize yields WS*attn —
    a better fp8 range (std ~1.3) for the aoutT staging; the Wo epilogue
    descales by 1/WS^2.
    """
    for hg in range(4):
        ETs = []
        for h4 in range(4):
            h = hg * 4 + h4
            pr = h4 * 32
            ET = ET_pool.tile([P, TSUB, SQ], f8, tag=f"ET{h4}")
            for grp in range(4):
                ps = ps_sc.tile([P, 2, SQ], f32, tag="sc")
                for c2 in range(2):
                    t_i = grp * 2 + c2
                    nc.tensor.matmul(
                        ps[:, c2],
                        kT8[pr:pr + 32, hg, :, t_i * P:(t_i + 1) * P],
                        qT8[pr:pr + 32, hg, :, :],
                        start=True, stop=True, perf_mode=DR,
                        tile_position=(pr, 0),
                    )
                nc.scalar.activation(
                    ET[:, grp * 2:(grp + 1) * 2, :], ps, AF.Exp, scale=SCALE)
            ETs.append(ET)
        for sc in range(QSUB):
            pv = ps_pv.tile([P, 4, P], f32, tag="pv")
            for h4 in range(4):
                for t2 in range(4):
                    nc.tensor.matmul(
                        pv[:, h4, 0:65],
                        ETs[h4][:, 2 * t2:2 * t2 + 2, sc * P:(sc + 1) * P],
                        v8[:, 2 * t2:2 * t2 + 2, hg * 4 + h4, :],
                        start=(t2 == 0), stop=(t2 == 3), perf_mode=DR,
                    )
            nc.vector.tensor_scalar(
                attn_un[:, sc, hg * 4:hg * 4 + 4, :],
                pv[:, :, 0:65], 1.0 / PVS, None, ALU.mult)


def _normalize(nc, sb_small, attn_un, attn8):
    """attn8[:, sc, :] = attn_un[.., 0:64] / attn_un[.., 64] per head."""
    for sc in range(QSUB):
        rec = sb_small.tile([P, H], f32, tag="nrm_rec")
        nc.vector.reciprocal(rec, attn_un[:, sc, :, 64])
        nc.vector.tensor_tensor(
            attn8[:, sc].rearrange("p (h w) -> p h w", h=H),
            attn_un[:, sc, :, 0:64],
            rec[:, :, None].to_broadcast((P, H, 64)),
            ALU.mult)


def _out_proj(nc, tc, w_dr, aoutT8, bias_row, ones_row, resid):
    """resid = resid + aout @ Wo + bias (bias via rank-1 bf16 matmul)."""
    with (
        tc.tile_pool(name="wo_w", bufs=5) as wpool,
        tc.tile_pool(name="wo_ps", bufs=3, space="PSUM") as ps_pool,
    ):
        halves = _load_w8(nc, wpool, w_dr)
        for sc in range(QSUB):
            for dh in range(2):
                ps = ps_pool.tile([P, 512], f32, tag="wo")
                for q in range(KP):
                    nc.tensor.matmul(
                        ps,
                        aoutT8[:, 2 * q:2 * q + 2, sc * P:(sc + 1) * P],
                        halves[q][:, :, dh * 512:(dh + 1) * 512],
                        start=(q == 0), stop=False, perf_mode=DR,
                    )
                nc.tensor.matmul(
                    ps, ones_row, bias_row[:, dh * 512:(dh + 1) * 512],
                    start=False, stop=True, skip_group_check=True,
                )
                sl = slice(dh * 512, (dh + 1) * 512)
                nc.vector.scalar_tensor_tensor(
                    resid[:, sc, sl], ps, 1.0 / (WS * WS), resid[:, sc, sl],
                    ALU.mult, ALU.add)


def _ln_transpose_q(nc, tc, sb_small, ident8, h_in, xnT8, eps_ap, eng,
                    xnT8_lo=None):
    """LN each of the 4 h-chunks and transpose into xnT8 [P, DSUB, SQ].

    If xnT8_lo is given, also write the fp8 quantization residual
    (bf16(xn) - fp8(xn)) for hi/lo double-fp8 matmuls.
    """
    with (
        tc.tile_pool(name="lnq", bufs=3) as xn_pool,
        tc.tile_pool(name="lnq_tr", bufs=3, space="PSUM") as ps_tr,
    ):
        for sc in range(QSUB):
            xn8 = xn_pool.tile([P, D], bf16, tag="xn")
            _ln_tile(nc, sb_small, h_in[:, sc], xn8, eps_ap)
            for a in range(2):
                tp = ps_tr.tile([P, 4, P], bf16, tag="tr_ps")
                for i in range(4):
                    nc.tensor.transpose(
                        tp[:, i],
                        xn8[:, a * 512 + i * P:a * 512 + (i + 1) * P], ident8)
                hi = xnT8[:, 4 * a:4 * a + 4, sc * P:(sc + 1) * P]
                if eng == "act":
                    nc.scalar.copy(hi, tp)
                else:
                    nc.vector.tensor_copy(hi, tp)
                if xnT8_lo is not None:
                    nc.vector.tensor_tensor(
                        xnT8_lo[:, 4 * a:4 * a + 4, sc * P:(sc + 1) * P],
                        tp, hi, ALU.subtract)


def _transpose_aout(nc, tc, ident8, attn8, aoutT8):
    with tc.tile_pool(name="aout_tr", bufs=3, space="PSUM") as ps_tr:
        for sc in range(QSUB):
            for a in range(2):
                _transpose4(
                    nc, ps_tr, ident8, attn8[:, sc, a * 512:(a + 1) * 512],
                    aoutT8[:, 4 * a:4 * a + 4, sc * P:(sc + 1) * P], "dve")


# --------------------------------------------------------------------------
# full program
# --------------------------------------------------------------------------

def build_nc(reps=1):
    nc = bacc.Bacc(None, target_bir_lowering=False, debug=False)

    x_dr = nc.dram_tensor("x", [S, D], f32, kind="ExternalInput")
    ctxT_dr = nc.dram_tensor("ctxT8", [D, T], f8, kind="ExternalInput")
    maskd_dr = nc.dram_tensor("maskd", [T], f32, kind="ExternalInput")
    wdr = {}
    for a in (1, 2):
        for nm in ("Wq", "Wk", "Wv", "Wo"):
            wdr[f"{nm}{a}"] = nc.dram_tensor(
                f"{nm}{a}", [D, D], f8, kind="ExternalInput")
    qb1_dr = nc.dram_tensor("qb1", [D], f32, kind="ExternalInput")
    kb1_dr = nc.dram_tensor("kb1", [D], f32, kind="ExternalInput")
    vb1_dr = nc.dram_tensor("vb1", [D], f32, kind="ExternalInput")
    qb2_dr = nc.dram_tensor("qb2", [D], f32, kind="ExternalInput")
    bo1_dr = nc.dram_tensor("bo1x", [D], bf16, kind="ExternalInput")
    bo2_dr = nc.dram_tensor("bo2x", [D], bf16, kind="ExternalInput")
    bff2_dr = nc.dram_tensor("bff2x", [D], bf16, kind="ExternalInput")
    wff1_dr = nc.dram_tensor("Wff1hl", [2 * D, 2 * FF], f8,
                             kind="ExternalInput")
    bff1h_dr = nc.dram_tensor("bff1h8", [FF], f32, kind="ExternalInput")
    bff1g_dr = nc.dram_tensor("bff1g", [FF], f32, kind="ExternalInput")
    wff2_dr = nc.dram_tensor("Wff2hl", [2 * FF, D], f8,
                             kind="ExternalInput")
    out_dr = nc.dram_tensor("out", [SQ, D], f32, kind="ExternalOutput")

    x_tiled = x_dr.rearrange("(ss p) d -> p ss d", p=P)

    with tile.TileContext(nc) as tc, contextlib.ExitStack() as es:
        const = es.enter_context(tc.tile_pool(name="const", bufs=1))
        sb_small = es.enter_context(tc.tile_pool(name="smalls", bufs=6))

        ident8 = const.tile([P, P], bf16)
        make_identity(nc, ident8)
        eps_ap = const.tile([P, 1], f32)
        nc.vector.memset(eps_ap, EPS)
        ones_row = const.tile([1, P], bf16)
        nc.vector.memset(ones_row, 1.0)
        bo1_row = const.tile([1, D], bf16)
        nc.sync.dma_start(bo1_row, bo1_dr[None, :])
        bo2_row = const.tile([1, D], bf16)
        nc.sync.dma_start(bo2_row, bo2_dr[None, :])
        bff2_row = const.tile([1, D], bf16)
        nc.sync.dma_start(bff2_row, bff2_dr[None, :])
        vb1_b = const.tile([P, D], f32)
        nc.sync.dma_start(vb1_b, vb1_dr[None, :].to_broadcast((P, D)))
        qb1_s = const.tile([P, DSUB], f32)
        nc.sync.dma_start(qb1_s, qb1_dr.rearrange("(c p) -> p c", p=P))
        kb1_s = const.tile([P, DSUB], f32)
        nc.sync.dma_start(kb1_s, kb1_dr.rearrange("(c p) -> p c", p=P))
        qb2_s = const.tile([P, DSUB], f32)
        nc.sync.dma_start(qb2_s, qb2_dr.rearrange("(c p) -> p c", p=P))
        bff1h_s = const.tile([P, FSUB], f32)
        nc.sync.dma_start(bff1h_s, bff1h_dr.rearrange("(c p) -> p c", p=P))
        bff1g_s = const.tile([P, FSUB], f32)
        nc.sync.dma_start(bff1g_s, bff1g_dr.rearrange("(c p) -> p c", p=P))
        maskd_s = const.tile([P, TSUB], f32)
        nc.sync.dma_start(maskd_s, maskd_dr.rearrange("(c p) -> p c", p=P))

        for _rep in range(reps):
            # Residual buffer, reused in place: x_q -> h1 -> h2 -> out.
            hbuf, free_hbuf = tc.tile([P, QSUB, D], f32, name="hbuf")
            for sc in range(QSUB):
                nc.sync.dma_start(hbuf[:, sc], x_tiled[:, sc])

            # Attention staging buffers shared by both blocks (freed after
            # wo2) — allocated first to keep pool lifetimes LIFO.
            attn_un, free_attn_un = tc.tile([P, QSUB, H, 65], f8,
                                            name="attn_un")
            attn8, free_attn8 = tc.tile([P, QSUB, D], bf16, name="attn8")

            # ---- Phase 1: LN1 + transpose (full S rows) ----
            xn1T, free_xn1T = tc.tile([P, DSUB, S], f8, name="xn1T")
            w1_es = contextlib.ExitStack()
            wpool1 = w1_es.enter_context(tc.tile_pool(name="w1", bufs=5))
            if _KSTOP >= 1:
             with (
                nc.named_scope("ln1"),
                tc.tile_pool(name="x_hi", bufs=3) as x_hi_pool,
                tc.tile_pool(name="xn1", bufs=3) as xn1_pool,
                tc.tile_pool(name="tr1_ps", bufs=3, space="PSUM") as ps_tr,
            ):
                for ss in range(SSUB):
                    if ss < QSUB:
                        xt = hbuf[:, ss]
                    else:
                        xt = x_hi_pool.tile([P, D], f32, tag="x_hi")
                        nc.sync.dma_start(xt, x_tiled[:, ss])
                    xn8 = xn1_pool.tile([P, D], bf16, tag="xn1")
                    _ln_tile(nc, sb_small, xt, xn8, eps_ap)
                    for a in range(2):
                        _transpose4(
                            nc, ps_tr, ident8, xn8[:, a * 512:(a + 1) * 512],
                            xn1T[:, 4 * a:4 * a + 4, ss * P:(ss + 1) * P],
                            "act")

            # ---- attention block 1 (self) ----
            q1T8, free_q1T = tc.tile([P, 4, 2, SQ], f8, name="q1T8")
            k1T8, free_k1T = tc.tile([P, 4, 2, T], f8, name="k1T8")
            v1, free_v1 = tc.tile([P, TSUB, H, 65], f8, name="v1")
            if _KSTOP >= 2:
             with (
                nc.named_scope("qkv1"),
                tc.tile_pool(name="qkv1_ps", bufs=6, space="PSUM") as ps_proj,
            ):
                _proj_qk(nc, wpool1, ps_proj, wdr["Wq1"], xn1T[:, :, :SQ],
                         q1T8, qb1_s, SQ, "dve")
                _proj_qk(nc, wpool1, ps_proj, wdr["Wk1"], xn1T,
                         k1T8, kb1_s, T, "act")
                _proj_v(nc, wpool1, ps_proj, wdr["Wv1"], xn1T, v1, vb1_b, None)
            if _KSTOP >= 3:
             with (
                nc.named_scope("attn1"),
                tc.tile_pool(name="ET1", bufs=5) as ET_pool,
                tc.tile_pool(name="sc1_ps", bufs=2, space="PSUM") as ps_sc,
                tc.tile_pool(name="pv1_ps", bufs=2, space="PSUM") as ps_pv,
            ):
                _attention(nc, ET_pool, ps_sc, ps_pv, sb_small, q1T8, k1T8,
                           v1, attn_un)
            free_v1(); free_k1T(); free_q1T()
            aout1T, free_aout1T = tc.tile([P, DSUB, SQ], f8, name="aout1T")
            if _KSTOP >= 4:
             with nc.named_scope("wo1"):
                _normalize(nc, sb_small, attn_un, attn8)
                _transpose_aout(nc, tc, ident8, attn8, aout1T)
                _out_proj(nc, tc, wdr["Wo1"], aout1T, bo1_row, ones_row, hbuf)
            free_aout1T(); w1_es.close(); free_xn1T()

            # ---- attention block 2 (cross) ----
            xn2T, free_xn2T = tc.tile([P, DSUB, SQ], f8, name="xn2T")
            w2_es = contextlib.ExitStack()
            wpool2 = w2_es.enter_context(tc.tile_pool(name="w2", bufs=5))
            q2T8, free_q2T = tc.tile([P, 4, 2, SQ], f8, name="q2T8")
            k2T8, free_k2T = tc.tile([P, 4, 2, T], f8, name="k2T8")
            v2, free_v2 = tc.tile([P, TSUB, H, 65], f8, name="v2")
            ctxT_sb, free_ctxT = tc.tile([P, DSUB, T], f8, name="ctxT_sb")
            _ctxT_t = ctxT_dr.rearrange("(ds p) t -> p ds t", p=P)
            for ds in range(DSUB):
                nc.sync.dma_start(ctxT_sb[:, ds], _ctxT_t[:, ds])
            if _KSTOP >= 5:
             with nc.named_scope("ln2"):
                _ln_transpose_q(nc, tc, sb_small, ident8, hbuf, xn2T, eps_ap,
                                "act")
            if _KSTOP >= 6:
             with (
                nc.named_scope("qkv2"),
                tc.tile_pool(name="qkv2_ps", bufs=6, space="PSUM") as ps_proj,
            ):
                _proj_qk(nc, wpool2, ps_proj, wdr["Wk2"], ctxT_sb,
                         k2T8, None, T, "act")
                _proj_v(nc, wpool2, ps_proj, wdr["Wv2"], ctxT_sb, v2, None,
                        maskd_s)
                _proj_qk(nc, wpool2, ps_proj, wdr["Wq2"], xn2T,
                         q2T8, qb2_s, SQ, "dve")
            free_ctxT()
            if _KSTOP >= 7:
             with (
                nc.named_scope("attn2"),
                tc.tile_pool(name="ET2", bufs=5) as ET_pool,
                tc.tile_pool(name="sc2_ps", bufs=2, space="PSUM") as ps_sc,
                tc.tile_pool(name="pv2_ps", bufs=2, space="PSUM") as ps_pv,
            ):
                _attention(nc, ET_pool, ps_sc, ps_pv, sb_small, q2T8, k2T8,
                           v2, attn_un)
            free_v2(); free_k2T(); free_q2T()
            aout2T, free_aout2T = tc.tile([P, DSUB, SQ], f8, name="aout2T")
            if _KSTOP >= 8:
             with nc.named_scope("wo2"):
                _normalize(nc, sb_small, attn_un, attn8)
                _transpose_aout(nc, tc, ident8, attn8, aout2T)
                _out_proj(nc, tc, wdr["Wo2"], aout2T, bo2_row, ones_row, hbuf)
            free_aout2T(); w2_es.close(); free_xn2T()
            free_attn8(); free_attn_un()

            # ---- GEGLU feed-forward ----
            xn3T, free_xn3T = tc.tile([P, 2, DSUB, SQ], f8, name="xn3T")
            if _KSTOP >= 9:
             with nc.named_scope("ln3"):
                _ln_transpose_q(nc, tc, sb_small, ident8, hbuf, xn3T[:, 0],
                                eps_ap, "act", xnT8_lo=xn3T[:, 1])

            mT8, free_mT8 = tc.tile([P, FSUB, SQ], f8, name="mT8")
            ff_es = contextlib.ExitStack()
            wff2_pool = ff_es.enter_context(tc.tile_pool(name="wff2", bufs=6))
            wff1_t = wff1_dr.rearrange("(ks p) f -> p ks f", p=P)
            if _KSTOP >= 10:
             with (
                nc.named_scope("ff1"),
                tc.tile_pool(name="wff1", bufs=4) as wff1_pool,
                tc.tile_pool(name="ff1_ps", bufs=6, space="PSUM") as ps_ff1,
                tc.tile_pool(name="hT", bufs=3) as hT_pool,
                tc.tile_pool(name="gT", bufs=3) as gT_pool,
            ):
                for grp in range(8):
                    wh = wff1_pool.tile([P, 2 * DSUB, 512], f8, tag="wff1h")
                    nc.sync.dma_start(
                        wh, wff1_t[:, :, grp * 512:(grp + 1) * 512])
                    wg = wff1_pool.tile([P, 2 * DSUB, 512], f8, tag="wff1g")
                    nc.sync.dma_start(
                        wg, wff1_t[:, :, FF + grp * 512:FF + (grp + 1) * 512])
                    for fi in range(4):
                        fc = grp * 4 + fi
                        hg = []
                        for part_i, wt in ((0, wh), (1, wg)):
                            ps = ps_ff1.tile([P, SQ], f32, tag="yT")
                            # chain 1: (A_q, B_q) x dup(x_hi_q) — exact W
                            for q in range(DSUB):
                                nc.tensor.matmul(
                                    ps,
                                    wt[:, 2 * q:2 * q + 2,
                                       fi * P:(fi + 1) * P],
                                    xn3T[:, 0, q:q + 1, :].to_broadcast(
                                        (P, 2, SQ)),
                                    start=(q == 0), stop=False,
                                    perf_mode=DR, skip_group_check=(q > 0))
                            # chain 2: (A_2q, A_2q+1) x (x_lo pair)
                            for q2 in range(KP):
                                nc.tensor.matmul(
                                    ps,
                                    wt[:, 4 * q2:4 * q2 + 4:2,
                                       fi * P:(fi + 1) * P],
                                    xn3T[:, 1, 2 * q2:2 * q2 + 2, :],
                                    start=False, stop=(q2 == KP - 1),
                                    perf_mode=DR, skip_group_check=True)
                            if part_i == 0:
                                hT = hT_pool.tile([P, SQ], bf16, tag="hT")
                                nc.vector.tensor_scalar(
                                    hT, ps, MS / WS, bff1h_s[:, fc:fc + 1],
                                    ALU.mult, ALU.add)
                                hg.append(hT)
                            else:
                                gT = gT_pool.tile([P, SQ], bf16, tag="gT")
                                nc.scalar.activation(
                                    gT, ps, AF.Gelu,
                                    bias=bff1g_s[:, fc:fc + 1], scale=1.0 / WS)
                                hg.append(gT)
                        nc.gpsimd.tensor_tensor(
                            mT8[:, fc, :], hg[0], hg[1], ALU.mult)

            wff2_t = wff2_dr.rearrange("(ks p) o -> p ks o", p=P)
            if _KSTOP >= 11:
             with (
                nc.named_scope("ff2"),
                tc.tile_pool(name="ff2_ps", bufs=1, space="PSUM") as ps_ff2,
            ):
                ps_o = [ps_ff2.tile([P, 512], f32, tag=f"o{i}", name=f"ps_o{i}")
                        for i in range(8)]
                for k in range(FSUB):
                    wt = wff2_pool.tile([P, 2, D], f8, tag="wff2")
                    nc.sync.dma_start(wt, wff2_t[:, 2 * k:2 * k + 2])
                    for sc in range(QSUB):
                        for dh in range(2):
                            # dup(m_k) x (A2_k, B2_k) — exact Wff2
                            nc.tensor.matmul(
                                ps_o[sc * 2 + dh],
                                mT8[:, k:k + 1,
                                    sc * P:(sc + 1) * P].to_broadcast(
                                        (P, 2, P)),
                                wt[:, :, dh * 512:(dh + 1) * 512],
                                start=(k == 0), stop=False, perf_mode=DR,
                                skip_group_check=(k > 0))
                for sc in range(QSUB):
                    for dh in range(2):
                        nc.tensor.matmul(
                            ps_o[sc * 2 + dh], ones_row,
                            bff2_row[:, dh * 512:(dh + 1) * 512],
                            start=False, stop=True, skip_group_check=True)
                        sl = slice(dh * 512, (dh + 1) * 512)
                        nc.vector.scalar_tensor_tensor(
                            hbuf[:, sc, sl], ps_o[sc * 2 + dh], 1.0 / (MS * WS),
                            hbuf[:, sc, sl], ALU.mult, ALU.add)
            ff_es.close(); free_mT8(); free_xn3T()

            nc.sync.dma_start(out_dr.rearrange("(ss p) d -> p ss d", p=P), hbuf)
            free_hbuf()

    nc.compile()
    return nc


# --------------------------------------------------------------------------
# host side
# --------------------------------------------------------------------------

_NC = None


def _get_nc():
    global _NC
    if _NC is None:
        _NC = build_nc()
    return _NC


def _f8(a):
    import ml_dtypes
    return np.ascontiguousarray(
        np.clip(np.asarray(a, np.float32), -240.0, 240.0).astype(
            ml_dtypes.float8_e4m3))


def _f8_hilo(Ws):
    """[Din, O] f32 (pre-scaled) -> [2*Din, O] f8, per-128-row-block
    interleaved (A_0, B_0, A_1, B_1, ...) where A = f8(W), B = f8(W - A)."""
    Ws = np.asarray(Ws, np.float32)
    A = _f8(Ws)
    Bq = _f8(Ws - A.astype(np.float32))
    Din, O = Ws.shape
    out = np.empty((2 * Din, O), dtype=A.dtype)
    nb = Din // P
    for q in range(nb):
        out[(2 * q) * P:(2 * q + 1) * P] = A[q * P:(q + 1) * P]
        out[(2 * q + 1) * P:(2 * q + 2) * P] = Bq[q * P:(q + 1) * P]
    return np.ascontiguousarray(out)


def _scores_perm():
    """Column permutation for Q/K weights: new col j <- old col d(j)."""
    j = np.arange(D)
    b, r = j // 256, j % 256
    i, p = r // 128, r % 128
    h4, dv32 = p // 32, p % 32
    return (4 * b + h4) * 64 + i * 32 + dv32


def _make_in_maps(inputs):
    f = np.float32
    hidden = np.asarray(inputs["hidden_states"], f)
    context = np.asarray(inputs["context"], f)
    mask = np.asarray(inputs["encoder_key_padding_mask"]).astype(f)
    g1, b1 = np.asarray(inputs["g1"], f), np.asarray(inputs["b1"], f)
    g2, b2 = np.asarray(inputs["g2"], f), np.asarray(inputs["b2"], f)
    g3, b3 = np.asarray(inputs["g3"], f), np.asarray(inputs["b3"], f)

    def fold(g, W):
        return g[:, None] * np.asarray(W, f)

    perm = _scores_perm()

    Wq1 = fold(g1, inputs["Wq1"])[:, perm]
    Wk1 = fold(g1, inputs["Wk1"])[:, perm]
    Wv1 = fold(g1, inputs["Wv1"])
    Wo1 = np.asarray(inputs["Wo1"], f)
    qb1 = (b1 @ np.asarray(inputs["Wq1"], f))[perm]
    kb1 = (b1 @ np.asarray(inputs["Wk1"], f))[perm]
    vb1 = b1 @ np.asarray(inputs["Wv1"], f)
    Wq2 = fold(g2, inputs["Wq2"])[:, perm]
    Wk2 = np.asarray(inputs["Wk2"], f)[:, perm]
    Wv2 = np.asarray(inputs["Wv2"], f)
    Wo2 = np.asarray(inputs["Wo2"], f)
    qb2 = (b2 @ np.asarray(inputs["Wq2"], f))[perm]
    Wff1 = fold(g3, inputs["Wff1"])
    bff1 = np.asarray(inputs["bff1"], f) + b3 @ np.asarray(inputs["Wff1"], f)
    Wff2 = np.asarray(inputs["Wff2"], f)

    import ml_dtypes
    bfa = lambda a: np.ascontiguousarray(np.asarray(a, f).astype(
        ml_dtypes.bfloat16))

    shared = {
        "Wq1": _f8(Wq1 * WS), "Wk1": _f8(Wk1 * WS), "Wv1": _f8(Wv1 * WS),
        "Wo1": _f8(Wo1 * WS),
        "qb1": np.ascontiguousarray(qb1), "kb1": np.ascontiguousarray(kb1),
        "vb1": np.ascontiguousarray(vb1),
        "bo1x": bfa(np.asarray(inputs["bo1"], f) * WS * WS),
        "Wq2": _f8(Wq2 * WS), "Wk2": _f8(Wk2 * WS), "Wv2": _f8(Wv2 * WS),
        "Wo2": _f8(Wo2 * WS),
        "qb2": np.ascontiguousarray(qb2),
        "bo2x": bfa(np.asarray(inputs["bo2"], f) * WS * WS),
        "Wff1hl": _f8_hilo(Wff1 * WS),
        "bff1h8": np.ascontiguousarray(bff1[:FF] * MS),
        "bff1g": np.ascontiguousarray(bff1[FF:]),
        "Wff2hl": _f8_hilo(Wff2 * WS),
        "bff2x": bfa(np.asarray(inputs["bff2"], f) * MS * WS),
    }

    in_maps = []
    for core in range(NCORES):
        b, q = core // 2, core % 2
        x = hidden[b] if q == 0 else np.roll(hidden[b], -SQ, axis=0)
        in_maps.append({
            **shared,
            "x": np.ascontiguousarray(x),
            "ctxT8": _f8(context[b].T),
            "maskd": np.ascontiguousarray(mask[b] / WS),
        })
    return in_maps


def run(inputs, **spmd_kwargs):
    res = run_bass_kernel_spmd(
        _get_nc(), _make_in_maps(inputs), core_ids=list(range(NCORES)),
        **spmd_kwargs)
    out = np.empty((B, S, D), np.float32)
    for core in range(NCORES):
        b, q = core // 2, core % 2
        out[b, q * SQ:(q + 1) * SQ] = res.results[core]["out"]
    return out, res


def kernel(**inputs):
    out, _ = run(inputs)
    return out


# revision 34
# speedup vs baseline: 1.2055x; 1.2055x over previous
"""BasicTransformerBlock (self-attn + cross-attn + GEGLU FF) on 8 TRN2 cores.

Sharding: sequence-parallel, no collectives. B=4 batches x 2 sequence-halves
= 8 shards; each core computes 512 query rows end-to-end, duplicating only
the (cheap) K/V projections for its batch. The host rolls each batch's
hidden_states so a core's query rows are always rows 0..511 — the kernel is
uniform SPMD.

v2: fp8 (e4m3, TRN float8e4) everywhere on the matmul path with DoubleRow
perf mode (contraction-256 per instruction):
  - weights host-quantized to fp8 at x64 scale; activations quantized on
    device (LN outputs std 1 -> direct; attn out via 1/256 staging scale)
  - Q/K projections use host-permuted weight columns so per-head dh=64 is
    laid out as [32 partitions x 2 pair-slots], letting the scores matmul
    run DoubleRow with 4 heads packed per 128 partitions (row tiling)
  - PV accumulates 4 heads per PSUM bank; softmax denominator via fused
    65th ones-column of V; batched normalize per s-chunk
  - residuals stay f32; out-proj/FF biases folded in as rank-1 bf16 matmuls
  - engine balance: exp/gelu/K-epilogues on ACT, LN-normalize + FF1-mult on
    GpSimd (Pool), everything PSUM-touching on DVE
"""

import contextlib
import os

import numpy as np

_KSTOP = int(os.environ.get("KSTOP", "99"))

import concourse.mybir as mybir
import concourse.tile as tile
from concourse import bacc
from concourse.bass_utils import run_bass_kernel_spmd
from concourse.masks import make_identity

P = 128
B, S, T, D, H, DH = 4, 1024, 1024, 1024, 16, 64
FF = 4 * D
SQ = 512                 # query rows per core
SCALE = DH ** -0.5
EPS = 1e-12
NCORES = 8

WS = 64.0                # weight fp8 scale
MS = 8.0                 # ff1 hidden (mT) fp8 scale
PVS = 256.0              # PV psum -> fp8 staging scale

f32 = mybir.dt.float32
f32r = mybir.dt.float32r
bf16 = mybir.dt.bfloat16
f8 = mybir.dt.float8e4
AF = mybir.ActivationFunctionType
ALU = mybir.AluOpType
DR = mybir.MatmulPerfMode.DoubleRow

DSUB = D // P            # 8
TSUB = T // P            # 8
SSUB = S // P            # 8
QSUB = SQ // P           # 4
FSUB = FF // P           # 32
KP = DSUB // 2           # 4 contraction k-pairs per D-deep matmul


# --------------------------------------------------------------------------
# device-program helpers
# --------------------------------------------------------------------------

def _ln_tile(nc, sb_small, x_ap, xn8_ap, eps_ap):
    """xn8 = (x - mean) * rsqrt(var + eps), written fp8 on Pool."""
    stats = sb_small.tile([P, D // 512, 6], f32, tag="ln_stats")
    for c in range(D // 512):
        nc.vector.bn_stats(stats[:, c], x_ap[:, c * 512:(c + 1) * 512])
    mv = sb_small.tile([P, 2], f32, tag="ln_mv")
    nc.vector.bn_aggr(mv, stats)
    std = sb_small.tile([P, 1], f32, tag="ln_std")
    nc.scalar.activation(std, mv[:, 1:2], AF.Sqrt, bias=eps_ap)
    rstd = sb_small.tile([P, 1], f32, tag="ln_rstd")
    nc.vector.reciprocal(rstd, std)
    nc.gpsimd.tensor_scalar(xn8_ap, x_ap, mv[:, 0:1], rstd,
                            ALU.subtract, ALU.mult)


def _transpose4(nc, ps_tr, ident8, src_f8, dst4, eng):
    """Transpose 4 adjacent [128,128] fp8 blocks; one merged copy to dst4.

    src_f8: [P, 512] fp8 AP (4 d-blocks); dst4: [P, 4, 128] fp8 AP.
    eng: 'act' or 'dve' for the PSUM->SBUF copy.
    """
    tp = ps_tr.tile([P, 4, P], bf16, tag="tr_ps")
    for i in range(4):
        nc.tensor.transpose(tp[:, i], src_f8[:, i * P:(i + 1) * P], ident8)
    if eng == "act":
        nc.scalar.copy(dst4, tp)
    else:
        nc.vector.tensor_copy(dst4, tp)


def _load_w8(nc, wpool, w_dr):
    """Stream a [D, O] fp8 weight as KP tiles [128, 2, O] (k = ks*128+p)."""
    O = w_dr.shape[1]
    parts = []
    for q in range(KP):
        wt = wpool.tile([P, 2, O], f8, tag="w")
        nc.sync.dma_start(
            wt, w_dr.rearrange("(ks p) o -> p ks o", p=P)[:, 2 * q:2 * q + 2])
        parts.append(wt)
    return parts


def _proj_qk_steps(nc, halves, ps_pool, rhsT, outT8, bias_s, ncols, eng):
    """Generator of per-PSUM-tile projection units (see _proj_qk)."""
    nhalf = ncols // 512
    for dsb in range(DSUB):
        for ch in range(nhalf):
            def emit(dsb=dsb, ch=ch):
                ps = ps_pool.tile([P, 512], f32, tag="proj")
                for q in range(KP):
                    nc.tensor.matmul(
                        ps,
                        halves[q][:, :, dsb * P:(dsb + 1) * P],
                        rhsT[:, 2 * q:2 * q + 2, ch * 512:(ch + 1) * 512],
                        start=(q == 0), stop=(q == KP - 1), perf_mode=DR,
                    )
                dst = outT8[:, dsb // 2, dsb % 2, ch * 512:(ch + 1) * 512]
                b = bias_s[:, dsb:dsb + 1] if bias_s is not None else 0.0
                if eng == "act":
                    nc.scalar.activation(dst, ps, AF.Identity, bias=b,
                                         scale=1.0 / WS)
                else:
                    nc.vector.tensor_scalar(dst, ps, 1.0 / WS, b,
                                            ALU.mult, ALU.add)
            yield emit


def _proj_qk(nc, halves, ps_pool, rhsT, outT8, bias_s, ncols, eng):
    """outT8[:, dsb//2, dsb%2, :] = (W.T @ xn)[d-chunk dsb] / WS + bias.

    rhsT: [P, DSUB, ncols] fp8; outT8: [P, 4, 2, ncols] fp8 (scores layout);
    bias_s: [P, DSUB] (host-permuted) or None. eng: epilogue engine.
    """
    for emit in _proj_qk_steps(nc, halves, ps_pool, rhsT, outT8, bias_s,
                               ncols, eng):
        emit()


def _proj_v_steps(nc, halves, ps_pool, lhsT8, v8, vb_b, maskd_s):
    """Generator of per-PSUM-tile V-projection units (see _proj_v)."""
    for ts in range(TSUB):
        for dh in range(2):
            def emit(ts=ts, dh=dh):
                ps = ps_pool.tile([P, 512], f32, tag="proj")
                for q in range(KP):
                    nc.tensor.matmul(
                        ps,
                        lhsT8[:, 2 * q:2 * q + 2, ts * P:(ts + 1) * P],
                        halves[q][:, :, dh * 512:(dh + 1) * 512],
                        start=(q == 0), stop=(q == KP - 1), perf_mode=DR,
                    )
                dst = v8[:, ts, dh * 8:(dh + 1) * 8, 0:64]
                src = ps.rearrange("p (h w) -> p h w", h=8)
                if maskd_s is None:
                    nc.vector.scalar_tensor_tensor(
                        dst, src, 1.0 / WS,
                        vb_b[:, dh * 512:(dh + 1) * 512].rearrange(
                            "p (h w) -> p h w", h=8),
                        ALU.mult, ALU.add)
                else:
                    nc.vector.tensor_scalar(dst, src, maskd_s[:, ts:ts + 1],
                                            None, ALU.mult)
            yield emit

    def emit_ones():
        if maskd_s is None:
            nc.vector.memset(v8[:, :, :, 64:65], 1.0 / WS)
        else:
            for ts in range(TSUB):
                nc.vector.tensor_copy(
                    v8[:, ts, :, 64],
                    maskd_s[:, ts:ts + 1].to_broadcast((P, H)))
    yield emit_ones


def _proj_v(nc, halves, ps_pool, lhsT8, v8, vb_b, maskd_s):
    """V[t, dv] natural fp8, per head, 65th column = ones (or mask)."""
    for emit in _proj_v_steps(nc, halves, ps_pool, lhsT8, v8, vb_b, maskd_s):
        emit()


def _attention(nc, ET_pool, ps_sc, ps_pv, sb_small, qT8, kT8, v8, attn_un,
               filler=None, filler_per_head=2):
    """scores^T (fp8 DoubleRow, 4 heads/row-tile) -> exp -> PV -> attn_un.

    attn_un: [P, QSUB, H, 65] fp8 = unnormalized PV / PVS. Column 64 holds
    sum(E)/WS (the ones-column of V is 1/WS), so normalize yields WS*attn —
    a better fp8 range (std ~1.3) for the aoutT staging; the Wo epilogue
    descales by 1/WS^2.
    """
    def pv_group(hg, ETs):
        for sc in range(QSUB):
            pv = ps_pv.tile([P, 4, P], f32, tag="pv")
            for h4 in range(4):
                for t2 in range(4):
                    nc.tensor.matmul(
                        pv[:, h4, 0:65],
                        ETs[h4][:, 2 * t2:2 * t2 + 2, sc * P:(sc + 1) * P],
                        v8[:, 2 * t2:2 * t2 + 2, hg * 4 + h4, :],
                        start=(t2 == 0), stop=(t2 == 3), perf_mode=DR,
                    )
            nc.vector.tensor_scalar(
                attn_un[:, sc, hg * 4:hg * 4 + 4, :],
                pv[:, :, 0:65], 1.0 / PVS, None, ALU.mult)

    prev = None
    for hg in range(4):
        ETs = []
        for h4 in range(4):
            h = hg * 4 + h4
            pr = h4 * 32
            ET = ET_pool.tile([P, TSUB, SQ], f8, tag=f"ET{h4}")
            for grp in range(4):
                ps = ps_sc.tile([P, 2, SQ], f32, tag="sc")
                for c2 in range(2):
                    t_i = grp * 2 + c2
                    nc.tensor.matmul(
                        ps[:, c2],
                        kT8[pr:pr + 32, hg, :, t_i * P:(t_i + 1) * P],
                        qT8[pr:pr + 32, hg, :, :],
                        start=True, stop=True, perf_mode=DR,
                        tile_position=(pr, 0),
                    )
                nc.scalar.activation(
                    ET[:, grp * 2:(grp + 1) * 2, :], ps, AF.Exp, scale=SCALE)
            ETs.append(ET)
            # Interleave a couple of independent filler units (e.g. the
            # cross-attention K2/V2 projections) per head so the PE queue
            # never piles long runs of filler in front of the next scores.
            if filler is not None:
                for _ in range(filler_per_head):
                    f = next(filler, None)
                    if f is not None:
                        f()
        # PV for the PREVIOUS head-group is emitted after this group's
        # scores so the scheduler keeps ACT (exp) fed with fresh scores
        # before draining PV matmuls.
        if prev is not None:
            pv_group(*prev)
        prev = (hg, ETs)
    pv_group(*prev)


def _normalize(nc, sb_small, attn_un, attn8):
    """attn8[:, sc, :] = attn_un[.., 0:64] / attn_un[.., 64] per head."""
    for sc in range(QSUB):
        rec = sb_small.tile([P, H], f32, tag="nrm_rec")
        nc.vector.reciprocal(rec, attn_un[:, sc, :, 64])
        nc.vector.tensor_tensor(
            attn8[:, sc].rearrange("p (h w) -> p h w", h=H),
            attn_un[:, sc, :, 0:64],
            rec[:, :, None].to_broadcast((P, H, 64)),
            ALU.mult)


def _out_proj(nc, tc, halves, aoutT8, bias_row, ones_row, resid):
    """resid = resid + aout @ Wo + bias (bias via rank-1 bf16 matmul)."""
    with (
        tc.tile_pool(name="wo_ps", bufs=3, space="PSUM") as ps_pool,
    ):
        for sc in range(QSUB):
            for dh in range(2):
                ps = ps_pool.tile([P, 512], f32, tag="wo")
                for q in range(KP):
                    nc.tensor.matmul(
                        ps,
                        aoutT8[:, 2 * q:2 * q + 2, sc * P:(sc + 1) * P],
                        halves[q][:, :, dh * 512:(dh + 1) * 512],
                        start=(q == 0), stop=False, perf_mode=DR,
                    )
                nc.tensor.matmul(
                    ps, ones_row, bias_row[:, dh * 512:(dh + 1) * 512],
                    start=False, stop=True, skip_group_check=True,
                )
                sl = slice(dh * 512, (dh + 1) * 512)
                nc.vector.scalar_tensor_tensor(
                    resid[:, sc, sl], ps, 1.0 / (WS * WS), resid[:, sc, sl],
                    ALU.mult, ALU.add)


def _ln_transpose_q(nc, tc, sb_small, ident8, h_in, xnT8, eps_ap, eng,
                    xnT8_lo=None):
    """LN each of the 4 h-chunks and transpose into xnT8 [P, DSUB, SQ].

    If xnT8_lo is given, also write the fp8 quantization residual
    (bf16(xn) - fp8(xn)) for hi/lo double-fp8 matmuls.
    """
    with (
        tc.tile_pool(name="lnq", bufs=3) as xn_pool,
        tc.tile_pool(name="lnq_tr", bufs=3, space="PSUM") as ps_tr,
    ):
        for sc in range(QSUB):
            xn8 = xn_pool.tile([P, D], bf16, tag="xn")
            _ln_tile(nc, sb_small, h_in[:, sc], xn8, eps_ap)
            for a in range(2):
                tp = ps_tr.tile([P, 4, P], bf16, tag="tr_ps")
                for i in range(4):
                    nc.tensor.transpose(
                        tp[:, i],
                        xn8[:, a * 512 + i * P:a * 512 + (i + 1) * P], ident8)
                hi = xnT8[:, 4 * a:4 * a + 4, sc * P:(sc + 1) * P]
                if eng == "act":
                    nc.scalar.copy(hi, tp)
                else:
                    nc.vector.tensor_copy(hi, tp)
                if xnT8_lo is not None:
                    nc.vector.tensor_tensor(
                        xnT8_lo[:, 4 * a:4 * a + 4, sc * P:(sc + 1) * P],
                        tp, hi, ALU.subtract)


def _transpose_aout(nc, tc, ident8, attn8, aoutT8):
    with tc.tile_pool(name="aout_tr", bufs=3, space="PSUM") as ps_tr:
        for sc in range(QSUB):
            for a in range(2):
                _transpose4(
                    nc, ps_tr, ident8, attn8[:, sc, a * 512:(a + 1) * 512],
                    aoutT8[:, 4 * a:4 * a + 4, sc * P:(sc + 1) * P], "act")


# --------------------------------------------------------------------------
# full program
# --------------------------------------------------------------------------

def build_nc(reps=1):
    nc = bacc.Bacc(None, target_bir_lowering=False, debug=False)

    x_dr = nc.dram_tensor("x", [S, D], f32, kind="ExternalInput")
    ctxT_dr = nc.dram_tensor("ctxT8", [D, T], f8, kind="ExternalInput")
    maskd_dr = nc.dram_tensor("maskd", [T], f32, kind="ExternalInput")
    wdr = {}
    for a in (1, 2):
        for nm in ("Wq", "Wk", "Wv", "Wo"):
            wdr[f"{nm}{a}"] = nc.dram_tensor(
                f"{nm}{a}", [D, D], f8, kind="ExternalInput")
    qb1_dr = nc.dram_tensor("qb1", [D], f32, kind="ExternalInput")
    kb1_dr = nc.dram_tensor("kb1", [D], f32, kind="ExternalInput")
    vb1_dr = nc.dram_tensor("vb1", [D], f32, kind="ExternalInput")
    qb2_dr = nc.dram_tensor("qb2", [D], f32, kind="ExternalInput")
    bo1_dr = nc.dram_tensor("bo1x", [D], bf16, kind="ExternalInput")
    bo2_dr = nc.dram_tensor("bo2x", [D], bf16, kind="ExternalInput")
    bff2_dr = nc.dram_tensor("bff2x", [D], bf16, kind="ExternalInput")
    wff1_dr = nc.dram_tensor("Wff1hl", [2 * D, 2 * FF], f8,
                             kind="ExternalInput")
    bff1h_dr = nc.dram_tensor("bff1h8", [FF], f32, kind="ExternalInput")
    bff1g_dr = nc.dram_tensor("bff1g", [FF], f32, kind="ExternalInput")
    wff2_dr = nc.dram_tensor("Wff2hl", [2 * FF, D], f8,
                             kind="ExternalInput")
    out_dr = nc.dram_tensor("out", [SQ, D], f32, kind="ExternalOutput")

    x_tiled = x_dr.rearrange("(ss p) d -> p ss d", p=P)

    with tile.TileContext(nc) as tc, contextlib.ExitStack() as es:
        const = es.enter_context(tc.tile_pool(name="const", bufs=1))
        sb_small = es.enter_context(tc.tile_pool(name="smalls", bufs=6))

        ident8 = const.tile([P, P], bf16)
        make_identity(nc, ident8)
        eps_ap = const.tile([P, 1], f32)
        nc.vector.memset(eps_ap, EPS)
        ones_row = const.tile([1, P], bf16)
        nc.vector.memset(ones_row, 1.0)
        bo1_row = const.tile([1, D], bf16)
        nc.sync.dma_start(bo1_row, bo1_dr[None, :])
        bo2_row = const.tile([1, D], bf16)
        nc.sync.dma_start(bo2_row, bo2_dr[None, :])
        bff2_row = const.tile([1, D], bf16)
        nc.sync.dma_start(bff2_row, bff2_dr[None, :])
        vb1_b = const.tile([P, D], f32)
        nc.sync.dma_start(vb1_b, vb1_dr[None, :].to_broadcast((P, D)))
        qb1_s = const.tile([P, DSUB], f32)
        nc.sync.dma_start(qb1_s, qb1_dr.rearrange("(c p) -> p c", p=P))
        kb1_s = const.tile([P, DSUB], f32)
        nc.sync.dma_start(kb1_s, kb1_dr.rearrange("(c p) -> p c", p=P))
        qb2_s = const.tile([P, DSUB], f32)
        nc.sync.dma_start(qb2_s, qb2_dr.rearrange("(c p) -> p c", p=P))
        bff1h_s = const.tile([P, FSUB], f32)
        nc.sync.dma_start(bff1h_s, bff1h_dr.rearrange("(c p) -> p c", p=P))
        bff1g_s = const.tile([P, FSUB], f32)
        nc.sync.dma_start(bff1g_s, bff1g_dr.rearrange("(c p) -> p c", p=P))
        maskd_s = const.tile([P, TSUB], f32)
        nc.sync.dma_start(maskd_s, maskd_dr.rearrange("(c p) -> p c", p=P))

        for _rep in range(reps):
            # Residual buffer, reused in place: x_q -> h1 -> h2 -> out.
            hbuf, free_hbuf = tc.tile([P, QSUB, D], f32, name="hbuf")
            with tc.high_priority():
                for sc in range(QSUB):
                    nc.sync.dma_start(hbuf[:, sc], x_tiled[:, sc])

            # Attention staging buffers shared by both blocks (freed after
            # wo2) — allocated first to keep pool lifetimes LIFO.
            attn_un, free_attn_un = tc.tile([P, QSUB, H, 65], f8,
                                            name="attn_un")
            attn8, free_attn8 = tc.tile([P, QSUB, D], bf16, name="attn8")

            # Cross-attention K2/V2 depend only on ctx — allocate their
            # tiles before block1 so the scheduler can hoist their
            # projections into attn1's ACT-bound window (no SBUF aliasing
            # anti-deps on block1 buffers).
            w2_es = contextlib.ExitStack()
            wpool2 = w2_es.enter_context(tc.tile_pool(name="w2", bufs=16))
            ps_kv2 = w2_es.enter_context(
                tc.tile_pool(name="kv2_ps", bufs=2, space="PSUM"))
            k2T8, free_k2T = tc.tile([P, 4, 2, T], f8, name="k2T8")
            v2, free_v2 = tc.tile([P, TSUB, H, 65], f8, name="v2")
            ctxT_sb, free_ctxT = tc.tile([P, DSUB, T], f8, name="ctxT_sb")
            _ctxT_t = ctxT_dr.rearrange("(ds p) t -> p ds t", p=P)
            for ds in range(DSUB):
                nc.sync.dma_start(ctxT_sb[:, ds], _ctxT_t[:, ds])
            # prefetch every attention weight now: dedicated buffers, early
            # DMA-queue slots (weight loads must never sit on the critical
            # path mid-kernel)
            w1_es = contextlib.ExitStack()
            wpool1 = w1_es.enter_context(tc.tile_pool(name="w1", bufs=16))
            w_q1 = _load_w8(nc, wpool1, wdr["Wq1"])
            w_k1 = _load_w8(nc, wpool1, wdr["Wk1"])
            w_v1 = _load_w8(nc, wpool1, wdr["Wv1"])
            w_o1 = _load_w8(nc, wpool1, wdr["Wo1"])
            w_k2 = _load_w8(nc, wpool2, wdr["Wk2"])
            w_v2 = _load_w8(nc, wpool2, wdr["Wv2"])
            w_q2 = _load_w8(nc, wpool2, wdr["Wq2"])
            w_o2 = _load_w8(nc, wpool2, wdr["Wo2"])

            # ---- Phase 1: LN1 + transpose (full S rows) ----
            xn1T, free_xn1T = tc.tile([P, DSUB, S], f8, name="xn1T")
            if _KSTOP >= 1:
             with (
                nc.named_scope("ln1"),
                tc.tile_pool(name="x_hi", bufs=2) as x_hi_pool,
                tc.tile_pool(name="xn1", bufs=3) as xn1_pool,
                tc.tile_pool(name="tr1_ps", bufs=3, space="PSUM") as ps_tr,
            ):
                for ss in range(SSUB):
                    if ss < QSUB:
                        xt = hbuf[:, ss]
                    else:
                        xt = x_hi_pool.tile([P, D], f32, tag="x_hi")
                        with tc.high_priority():
                            nc.sync.dma_start(xt, x_tiled[:, ss])
                    xn8 = xn1_pool.tile([P, D], bf16, tag="xn1")
                    _ln_tile(nc, sb_small, xt, xn8, eps_ap)
                    for a in range(2):
                        _transpose4(
                            nc, ps_tr, ident8, xn8[:, a * 512:(a + 1) * 512],
                            xn1T[:, 4 * a:4 * a + 4, ss * P:(ss + 1) * P],
                            "act")

            # ---- attention block 1 (self) ----
            q1T8, free_q1T = tc.tile([P, 4, 2, SQ], f8, name="q1T8")
            k1T8, free_k1T = tc.tile([P, 4, 2, T], f8, name="k1T8")
            v1, free_v1 = tc.tile([P, TSUB, H, 65], f8, name="v1")
            if _KSTOP >= 2:
             with (
                nc.named_scope("qkv1"),
                tc.tile_pool(name="qkv1_ps", bufs=6, space="PSUM") as ps_proj,
            ):
                _proj_qk(nc, w_q1, ps_proj, xn1T[:, :, :SQ],
                         q1T8, qb1_s, SQ, "dve")
                _proj_qk(nc, w_k1, ps_proj, xn1T,
                         k1T8, kb1_s, T, "act")
                _proj_v(nc, w_v1, ps_proj, xn1T, v1, vb1_b, None)
            kv2_filler = None
            if _KSTOP >= 6:
                import itertools
                kv2_filler = itertools.chain(
                    _proj_qk_steps(nc, w_k2, ps_kv2, ctxT_sb, k2T8, None, T,
                                   "dve"),
                    _proj_v_steps(nc, w_v2, ps_kv2, ctxT_sb, v2, None,
                                  maskd_s))
            if _KSTOP >= 3:
             with (
                nc.named_scope("attn1"),
                tc.tile_pool(name="ET1", bufs=2) as ET_pool,
                tc.tile_pool(name="sc1_ps", bufs=2, space="PSUM") as ps_sc,
                tc.tile_pool(name="pv1_ps", bufs=2, space="PSUM") as ps_pv,
            ):
                _attention(nc, ET_pool, ps_sc, ps_pv, sb_small, q1T8, k1T8,
                           v1, attn_un, filler=kv2_filler)
            free_v1(); free_k1T(); free_q1T()
            aout1T, free_aout1T = tc.tile([P, DSUB, SQ], f8, name="aout1T")
            if _KSTOP >= 4:
             with nc.named_scope("wo1"):
                _normalize(nc, sb_small, attn_un, attn8)
                _transpose_aout(nc, tc, ident8, attn8, aout1T)
                _out_proj(nc, tc, w_o1, aout1T, bo1_row, ones_row, hbuf)
            free_aout1T(); free_xn1T(); w1_es.close()

            # ---- attention block 2 (cross) ----
            xn2T, free_xn2T = tc.tile([P, DSUB, SQ], f8, name="xn2T")
            q2T8, free_q2T = tc.tile([P, 4, 2, SQ], f8, name="q2T8")
            if _KSTOP >= 5:
             with nc.named_scope("ln2"):
                _ln_transpose_q(nc, tc, sb_small, ident8, hbuf, xn2T, eps_ap,
                                "act")
            if _KSTOP >= 6:
             with nc.named_scope("qkv2"):
                if kv2_filler is not None:
                    for f in kv2_filler:
                        f()
                _proj_qk(nc, w_q2, ps_kv2, xn2T,
                         q2T8, qb2_s, SQ, "act")
            if _KSTOP >= 7:
             with (
                nc.named_scope("attn2"),
                tc.tile_pool(name="ET2", bufs=2) as ET_pool,
                tc.tile_pool(name="sc2_ps", bufs=2, space="PSUM") as ps_sc,
                tc.tile_pool(name="pv2_ps", bufs=2, space="PSUM") as ps_pv,
            ):
                _attention(nc, ET_pool, ps_sc, ps_pv, sb_small, q2T8, k2T8,
                           v2, attn_un)
            free_q2T()
            aout2T, free_aout2T = tc.tile([P, DSUB, SQ], f8, name="aout2T")
            if _KSTOP >= 8:
             with nc.named_scope("wo2"):
                _normalize(nc, sb_small, attn_un, attn8)
                _transpose_aout(nc, tc, ident8, attn8, aout2T)
                _out_proj(nc, tc, w_o2, aout2T, bo2_row, ones_row, hbuf)
            free_aout2T(); free_xn2T()
            free_ctxT(); free_v2(); free_k2T(); w2_es.close()
            free_attn8(); free_attn_un()

            # ---- GEGLU feed-forward ----
            xn3T, free_xn3T = tc.tile([P, 2, DSUB, SQ], f8, name="xn3T")
            if _KSTOP >= 9:
             with nc.named_scope("ln3"):
                _ln_transpose_q(nc, tc, sb_small, ident8, hbuf, xn3T[:, 0],
                                eps_ap, "act", xnT8_lo=xn3T[:, 1])

            mT8, free_mT8 = tc.tile([P, FSUB, SQ], f8, name="mT8")
            ff_es = contextlib.ExitStack()
            wff2_pool = ff_es.enter_context(tc.tile_pool(name="wff2", bufs=6))
            wff1_t = wff1_dr.rearrange("(ks p) f -> p ks f", p=P)
            if _KSTOP >= 10:
             with (
                nc.named_scope("ff1"),
                tc.tile_pool(name="wff1", bufs=4) as wff1_pool,
                tc.tile_pool(name="ff1_ps", bufs=6, space="PSUM") as ps_ff1,
                tc.tile_pool(name="hT", bufs=3) as hT_pool,
                tc.tile_pool(name="gT", bufs=3) as gT_pool,
            ):
                for grp in range(8):
                    wh = wff1_pool.tile([P, 2 * DSUB, 512], f8, tag="wff1h")
                    nc.sync.dma_start(
                        wh, wff1_t[:, :, grp * 512:(grp + 1) * 512])
                    wg = wff1_pool.tile([P, 2 * DSUB, 512], f8, tag="wff1g")
                    nc.sync.dma_start(
                        wg, wff1_t[:, :, FF + grp * 512:FF + (grp + 1) * 512])
                    for fi in range(4):
                        fc = grp * 4 + fi
                        hg = []
                        for part_i, wt in ((0, wh), (1, wg)):
                            ps = ps_ff1.tile([P, SQ], f32, tag="yT")
                            if part_i == 0:
                                # h-part: (A_q, B_q) x dup(x_hi_q) — exact W
                                for q in range(DSUB):
                                    nc.tensor.matmul(
                                        ps,
                                        wt[:, 2 * q:2 * q + 2,
                                           fi * P:(fi + 1) * P],
                                        xn3T[:, 0, q:q + 1, :].to_broadcast(
                                            (P, 2, SQ)),
                                        start=(q == 0), stop=False,
                                        perf_mode=DR,
                                        skip_group_check=(q > 0))
                                # + (A_2q, A_2q+1) x (x_lo pair)
                                for q2 in range(KP):
                                    nc.tensor.matmul(
                                        ps,
                                        wt[:, 4 * q2:4 * q2 + 4:2,
                                           fi * P:(fi + 1) * P],
                                        xn3T[:, 1, 2 * q2:2 * q2 + 2, :],
                                        start=False, stop=(q2 == KP - 1),
                                        perf_mode=DR, skip_group_check=True)
                            else:
                                # gate: plain fp8 (A slots, x_hi pairs) —
                                # gelu' squashes the quantization error
                                for q2 in range(KP):
                                    nc.tensor.matmul(
                                        ps,
                                        wt[:, 4 * q2:4 * q2 + 4:2,
                                           fi * P:(fi + 1) * P],
                                        xn3T[:, 0, 2 * q2:2 * q2 + 2, :],
                                        start=(q2 == 0), stop=(q2 == KP - 1),
                                        perf_mode=DR,
                                        skip_group_check=(q2 > 0))
                            if part_i == 0:
                                hT = hT_pool.tile([P, SQ], bf16, tag="hT")
                                nc.vector.tensor_scalar(
                                    hT, ps, MS / WS, bff1h_s[:, fc:fc + 1],
                                    ALU.mult, ALU.add)
                                hg.append(hT)
                            else:
                                gT = gT_pool.tile([P, SQ], bf16, tag="gT")
                                nc.scalar.activation(
                                    gT, ps, AF.Gelu,
                                    bias=bff1g_s[:, fc:fc + 1], scale=1.0 / WS)
                                hg.append(gT)
                        nc.gpsimd.tensor_tensor(
                            mT8[:, fc, :], hg[0], hg[1], ALU.mult)

            wff2_t = wff2_dr.rearrange("(ks p) o -> p ks o", p=P)
            if _KSTOP >= 11:
             with (
                nc.named_scope("ff2"),
                tc.tile_pool(name="ff2_ps", bufs=1, space="PSUM") as ps_ff2,
            ):
                ps_o = [ps_ff2.tile([P, 512], f32, tag=f"o{i}", name=f"ps_o{i}")
                        for i in range(8)]
                for sc in range(QSUB):
                    for dh in range(2):
                        nc.tensor.matmul(
                            ps_o[sc * 2 + dh], ones_row,
                            bff2_row[:, dh * 512:(dh + 1) * 512],
                            start=True, stop=False, skip_group_check=True)
                for k in range(FSUB):
                    wt = wff2_pool.tile([P, 2, D], f8, tag="wff2")
                    nc.sync.dma_start(wt, wff2_t[:, 2 * k:2 * k + 2])
                    for sc in range(QSUB):
                        for dh in range(2):
                            # dup(m_k) x (A2_k, B2_k) — exact Wff2
                            nc.tensor.matmul(
                                ps_o[sc * 2 + dh],
                                mT8[:, k:k + 1,
                                    sc * P:(sc + 1) * P].to_broadcast(
                                        (P, 2, P)),
                                wt[:, :, dh * 512:(dh + 1) * 512],
                                start=False, stop=(k == FSUB - 1),
                                perf_mode=DR, skip_group_check=True)
                for sc in range(QSUB):
                    for dh in range(2):
                        sl = slice(dh * 512, (dh + 1) * 512)
                        nc.vector.scalar_tensor_tensor(
                            hbuf[:, sc, sl], ps_o[sc * 2 + dh], 1.0 / (MS * WS),
                            hbuf[:, sc, sl], ALU.mult, ALU.add)
            ff_es.close(); free_mT8(); free_xn3T()

            out_t = out_dr.rearrange("(ss p) d -> p ss d", p=P)
            for sc in range(QSUB):
                nc.sync.dma_start(out_t[:, sc], hbuf[:, sc])
            free_hbuf()

    nc.compile()
    return nc


# --------------------------------------------------------------------------
# host side
# --------------------------------------------------------------------------

_NC = None


def _get_nc():
    global _NC
    if _NC is None:
        _NC = build_nc()
    return _NC


def _f8(a):
    import ml_dtypes
    return np.ascontiguousarray(
        np.clip(np.asarray(a, np.float32), -240.0, 240.0).astype(
            ml_dtypes.float8_e4m3))


def _f8_hilo(Ws):
    """[Din, O] f32 (pre-scaled) -> [2*Din, O] f8, per-128-row-block
    interleaved (A_0, B_0, A_1, B_1, ...) where A = f8(W), B = f8(W - A)."""
    Ws = np.asarray(Ws, np.float32)
    A = _f8(Ws)
    Bq = _f8(Ws - A.astype(np.float32))
    Din, O = Ws.shape
    out = np.empty((2 * Din, O), dtype=A.dtype)
    nb = Din // P
    for q in range(nb):
        out[(2 * q) * P:(2 * q + 1) * P] = A[q * P:(q + 1) * P]
        out[(2 * q + 1) * P:(2 * q + 2) * P] = Bq[q * P:(q + 1) * P]
    return np.ascontiguousarray(out)


def _scores_perm():
    """Column permutation for Q/K weights: new col j <- old col d(j)."""
    j = np.arange(D)
    b, r = j // 256, j % 256
    i, p = r // 128, r % 128
    h4, dv32 = p // 32, p % 32
    return (4 * b + h4) * 64 + i * 32 + dv32


def _make_in_maps(inputs):
    f = np.float32
    hidden = np.asarray(inputs["hidden_states"], f)
    context = np.asarray(inputs["context"], f)
    mask = np.asarray(inputs["encoder_key_padding_mask"]).astype(f)
    g1, b1 = np.asarray(inputs["g1"], f), np.asarray(inputs["b1"], f)
    g2, b2 = np.asarray(inputs["g2"], f), np.asarray(inputs["b2"], f)
    g3, b3 = np.asarray(inputs["g3"], f), np.asarray(inputs["b3"], f)

    def fold(g, W):
        return g[:, None] * np.asarray(W, f)

    perm = _scores_perm()

    Wq1 = fold(g1, inputs["Wq1"])[:, perm]
    Wk1 = fold(g1, inputs["Wk1"])[:, perm]
    Wv1 = fold(g1, inputs["Wv1"])
    Wo1 = np.asarray(inputs["Wo1"], f)
    qb1 = (b1 @ np.asarray(inputs["Wq1"], f))[perm]
    kb1 = (b1 @ np.asarray(inputs["Wk1"], f))[perm]
    vb1 = b1 @ np.asarray(inputs["Wv1"], f)
    Wq2 = fold(g2, inputs["Wq2"])[:, perm]
    Wk2 = np.asarray(inputs["Wk2"], f)[:, perm]
    Wv2 = np.asarray(inputs["Wv2"], f)
    Wo2 = np.asarray(inputs["Wo2"], f)
    qb2 = (b2 @ np.asarray(inputs["Wq2"], f))[perm]
    Wff1 = fold(g3, inputs["Wff1"])
    bff1 = np.asarray(inputs["bff1"], f) + b3 @ np.asarray(inputs["Wff1"], f)
    Wff2 = np.asarray(inputs["Wff2"], f)

    import ml_dtypes
    bfa = lambda a: np.ascontiguousarray(np.asarray(a, f).astype(
        ml_dtypes.bfloat16))

    shared = {
        "Wq1": _f8(Wq1 * WS), "Wk1": _f8(Wk1 * WS), "Wv1": _f8(Wv1 * WS),
        "Wo1": _f8(Wo1 * WS),
        "qb1": np.ascontiguousarray(qb1), "kb1": np.ascontiguousarray(kb1),
        "vb1": np.ascontiguousarray(vb1),
        "bo1x": bfa(np.asarray(inputs["bo1"], f) * WS * WS),
        "Wq2": _f8(Wq2 * WS), "Wk2": _f8(Wk2 * WS), "Wv2": _f8(Wv2 * WS),
        "Wo2": _f8(Wo2 * WS),
        "qb2": np.ascontiguousarray(qb2),
        "bo2x": bfa(np.asarray(inputs["bo2"], f) * WS * WS),
        "Wff1hl": _f8_hilo(Wff1 * WS),
        "bff1h8": np.ascontiguousarray(bff1[:FF] * MS),
        "bff1g": np.ascontiguousarray(bff1[FF:]),
        "Wff2hl": _f8_hilo(Wff2 * WS),
        "bff2x": bfa(np.asarray(inputs["bff2"], f) * MS * WS),
    }

    in_maps = []
    for core in range(NCORES):
        b, q = core // 2, core % 2
        x = hidden[b] if q == 0 else np.roll(hidden[b], -SQ, axis=0)
        in_maps.append({
            **shared,
            "x": np.ascontiguousarray(x),
            "ctxT8": _f8(context[b].T),
            "maskd": np.ascontiguousarray(mask[b] / WS),
        })
    return in_maps


def run(inputs, **spmd_kwargs):
    res = run_bass_kernel_spmd(
        _get_nc(), _make_in_maps(inputs), core_ids=list(range(NCORES)),
        **spmd_kwargs)
    out = np.empty((B, S, D), np.float32)
    for core in range(NCORES):
        b, q = core // 2, core % 2
        out[b, q * SQ:(q + 1) * SQ] = res.results[core]["out"]
    return out, res


def kernel(**inputs):
    out, _ = run(inputs)
    return out


# revision 44
# speedup vs baseline: 1.2338x; 1.0235x over previous
"""BasicTransformerBlock (self-attn + cross-attn + GEGLU FF) on 8 TRN2 cores.

Sharding: sequence-parallel, no collectives. B=4 batches x 2 sequence-halves
= 8 shards; each core computes 512 query rows end-to-end, duplicating only
the (cheap) K/V projections for its batch. The host rolls each batch's
hidden_states so a core's query rows are always rows 0..511 — the kernel is
uniform SPMD.

v2: fp8 (e4m3, TRN float8e4) everywhere on the matmul path with DoubleRow
perf mode (contraction-256 per instruction):
  - weights host-quantized to fp8 at x64 scale; activations quantized on
    device (LN outputs std 1 -> direct; attn out via 1/256 staging scale)
  - Q/K projections use host-permuted weight columns so per-head dh=64 is
    laid out as [32 partitions x 2 pair-slots], letting the scores matmul
    run DoubleRow with 4 heads packed per 128 partitions (row tiling)
  - PV accumulates 4 heads per PSUM bank; softmax denominator via fused
    65th ones-column of V; batched normalize per s-chunk
  - residuals stay f32; out-proj/FF biases folded in as rank-1 bf16 matmuls
  - engine balance: exp/gelu/K-epilogues on ACT, LN-normalize + FF1-mult on
    GpSimd (Pool), everything PSUM-touching on DVE
"""

import contextlib
import os

import numpy as np

_KSTOP = int(os.environ.get("KSTOP", "99"))

import concourse.mybir as mybir
import concourse.tile as tile
from concourse import bacc
from concourse.bass_utils import run_bass_kernel_spmd
from concourse.masks import make_identity

P = 128
B, S, T, D, H, DH = 4, 1024, 1024, 1024, 16, 64
FF = 4 * D
SQ = 512                 # query rows per core
SCALE = DH ** -0.5
EPS = 1e-12
NCORES = 8

WS = 64.0                # weight fp8 scale
MS = 8.0                 # ff1 hidden (mT) fp8 scale
PVS = 256.0              # PV psum -> fp8 staging scale

f32 = mybir.dt.float32
f32r = mybir.dt.float32r
bf16 = mybir.dt.bfloat16
f8 = mybir.dt.float8e4
AF = mybir.ActivationFunctionType
ALU = mybir.AluOpType
DR = mybir.MatmulPerfMode.DoubleRow

DSUB = D // P            # 8
TSUB = T // P            # 8
SSUB = S // P            # 8
QSUB = SQ // P           # 4
FSUB = FF // P           # 32
KP = DSUB // 2           # 4 contraction k-pairs per D-deep matmul


# --------------------------------------------------------------------------
# device-program helpers
# --------------------------------------------------------------------------

def _ln_tile(nc, sb_small, x_ap, xn8_ap, eps_ap):
    """xn8 = (x - mean) * rsqrt(var + eps), written fp8 on Pool."""
    stats = sb_small.tile([P, D // 512, 6], f32, tag="ln_stats")
    for c in range(D // 512):
        nc.vector.bn_stats(stats[:, c], x_ap[:, c * 512:(c + 1) * 512])
    mv = sb_small.tile([P, 2], f32, tag="ln_mv")
    nc.vector.bn_aggr(mv, stats)
    std = sb_small.tile([P, 1], f32, tag="ln_std")
    nc.scalar.activation(std, mv[:, 1:2], AF.Sqrt, bias=eps_ap)
    rstd = sb_small.tile([P, 1], f32, tag="ln_rstd")
    nc.vector.reciprocal(rstd, std)
    nc.gpsimd.tensor_scalar(xn8_ap, x_ap, mv[:, 0:1], rstd,
                            ALU.subtract, ALU.mult)


def _transpose4(nc, ps_tr, ident8, src_f8, dst4, eng):
    """Transpose 4 adjacent [128,128] fp8 blocks; one merged copy to dst4.

    src_f8: [P, 512] fp8 AP (4 d-blocks); dst4: [P, 4, 128] fp8 AP.
    eng: 'act' or 'dve' for the PSUM->SBUF copy.
    """
    tp = ps_tr.tile([P, 4, P], bf16, tag="tr_ps")
    for i in range(4):
        nc.tensor.transpose(tp[:, i], src_f8[:, i * P:(i + 1) * P], ident8)
    if eng == "act":
        nc.scalar.copy(dst4, tp)
    else:
        nc.vector.tensor_copy(dst4, tp)


def _load_w8(nc, wpool, w_dr):
    """Stream a [D, O] fp8 weight as KP tiles [128, 2, O] (k = ks*128+p)."""
    O = w_dr.shape[1]
    parts = []
    for q in range(KP):
        wt = wpool.tile([P, 2, O], f8, tag="w")
        nc.sync.dma_start(
            wt, w_dr.rearrange("(ks p) o -> p ks o", p=P)[:, 2 * q:2 * q + 2])
        parts.append(wt)
    return parts


def _proj_qk_steps(nc, halves, ps_pool, rhsT, outT8, bias_s, ncols, eng):
    """Generator of per-PSUM-tile projection units (see _proj_qk)."""
    nhalf = ncols // 512
    for dsb in range(DSUB):
        for ch in range(nhalf):
            def emit(dsb=dsb, ch=ch):
                ps = ps_pool.tile([P, 512], f32, tag="proj")
                for q in range(KP):
                    nc.tensor.matmul(
                        ps,
                        halves[q][:, :, dsb * P:(dsb + 1) * P],
                        rhsT[:, 2 * q:2 * q + 2, ch * 512:(ch + 1) * 512],
                        start=(q == 0), stop=(q == KP - 1), perf_mode=DR,
                    )
                dst = outT8[:, dsb // 2, dsb % 2, ch * 512:(ch + 1) * 512]
                b = bias_s[:, dsb:dsb + 1] if bias_s is not None else 0.0
                if eng == "act_ch0" :
                    eng2 = "act" if ch == 0 else "dve"
                else:
                    eng2 = eng
                if eng2 == "act":
                    nc.scalar.activation(dst, ps, AF.Identity, bias=b,
                                         scale=1.0 / WS)
                else:
                    nc.vector.tensor_scalar(dst, ps, 1.0 / WS, b,
                                            ALU.mult, ALU.add)
            yield emit


def _proj_qk(nc, halves, ps_pool, rhsT, outT8, bias_s, ncols, eng):
    """outT8[:, dsb//2, dsb%2, :] = (W.T @ xn)[d-chunk dsb] / WS + bias.

    rhsT: [P, DSUB, ncols] fp8; outT8: [P, 4, 2, ncols] fp8 (scores layout);
    bias_s: [P, DSUB] (host-permuted) or None. eng: epilogue engine.
    """
    for emit in _proj_qk_steps(nc, halves, ps_pool, rhsT, outT8, bias_s,
                               ncols, eng):
        emit()


def _proj_v_steps(nc, halves, ps_pool, lhsT8, v8, vb_b, maskd_s):
    """Generator of per-PSUM-tile V-projection units (see _proj_v)."""
    for ts in range(TSUB):
        for dh in range(2):
            def emit(ts=ts, dh=dh):
                ps = ps_pool.tile([P, 512], f32, tag="proj")
                for q in range(KP):
                    nc.tensor.matmul(
                        ps,
                        lhsT8[:, 2 * q:2 * q + 2, ts * P:(ts + 1) * P],
                        halves[q][:, :, dh * 512:(dh + 1) * 512],
                        start=(q == 0), stop=(q == KP - 1), perf_mode=DR,
                    )
                dst = v8[:, ts, dh * 8:(dh + 1) * 8, 0:64]
                src = ps.rearrange("p (h w) -> p h w", h=8)
                if maskd_s is None:
                    nc.vector.scalar_tensor_tensor(
                        dst, src, 1.0 / WS,
                        vb_b[:, dh * 512:(dh + 1) * 512].rearrange(
                            "p (h w) -> p h w", h=8),
                        ALU.mult, ALU.add)
                else:
                    nc.vector.tensor_scalar(dst, src, maskd_s[:, ts:ts + 1],
                                            None, ALU.mult)
            yield emit

    def emit_ones():
        if maskd_s is None:
            nc.vector.memset(v8[:, :, :, 64:65], 1.0 / WS)
        else:
            for ts in range(TSUB):
                nc.vector.tensor_copy(
                    v8[:, ts, :, 64],
                    maskd_s[:, ts:ts + 1].to_broadcast((P, H)))
    yield emit_ones


def _proj_v(nc, halves, ps_pool, lhsT8, v8, vb_b, maskd_s):
    """V[t, dv] natural fp8, per head, 65th column = ones (or mask)."""
    for emit in _proj_v_steps(nc, halves, ps_pool, lhsT8, v8, vb_b, maskd_s):
        emit()


def _attention(nc, ET_pool, ps_sc, ps_pv, sb_small, qT8, kT8, v8, attn_un,
               filler=None, filler_per_head=2):
    """scores^T (fp8 DoubleRow, 4 heads/row-tile) -> exp -> PV -> attn_un.

    attn_un: [P, QSUB, H, 65] fp8 = unnormalized PV / PVS. Column 64 holds
    sum(E)/WS (the ones-column of V is 1/WS), so normalize yields WS*attn —
    a better fp8 range (std ~1.3) for the aoutT staging; the Wo epilogue
    descales by 1/WS^2.
    """
    def pv_group(hg, ETs):
        for sc in range(QSUB):
            pv = ps_pv.tile([P, 4, P], f32, tag="pv")
            for h4 in range(4):
                for t2 in range(4):
                    nc.tensor.matmul(
                        pv[:, h4, 0:65],
                        ETs[h4][:, 2 * t2:2 * t2 + 2, sc * P:(sc + 1) * P],
                        v8[:, 2 * t2:2 * t2 + 2, hg * 4 + h4, :],
                        start=(t2 == 0), stop=(t2 == 3), perf_mode=DR,
                    )
            nc.vector.tensor_scalar(
                attn_un[:, sc, hg * 4:hg * 4 + 4, :],
                pv[:, :, 0:65], 1.0 / PVS, None, ALU.mult)

    prev = None
    for hg in range(4):
        ETs = []
        for h4 in range(4):
            h = hg * 4 + h4
            pr = h4 * 32
            ET = ET_pool.tile([P, TSUB, SQ], f8, tag=f"ET{h4}")
            for grp in range(4):
                ps = ps_sc.tile([P, 2, SQ], f32, tag="sc")
                for c2 in range(2):
                    t_i = grp * 2 + c2
                    nc.tensor.matmul(
                        ps[:, c2],
                        kT8[pr:pr + 32, hg, :, t_i * P:(t_i + 1) * P],
                        qT8[pr:pr + 32, hg, :, :],
                        start=True, stop=True, perf_mode=DR,
                        tile_position=(pr, 0),
                    )
                nc.scalar.activation(
                    ET[:, grp * 2:(grp + 1) * 2, :], ps, AF.Exp, scale=SCALE)
            ETs.append(ET)
            # Interleave a couple of independent filler units (e.g. the
            # cross-attention K2/V2 projections) per head so the PE queue
            # never piles long runs of filler in front of the next scores.
            if filler is not None:
                for _ in range(filler_per_head):
                    f = next(filler, None)
                    if f is not None:
                        f()
        # PV for the PREVIOUS head-group is emitted after this group's
        # scores so the scheduler keeps ACT (exp) fed with fresh scores
        # before draining PV matmuls.
        if prev is not None:
            pv_group(*prev)
        prev = (hg, ETs)
    pv_group(*prev)


def _normalize(nc, sb_small, attn_un, attn8):
    """attn8[:, sc, :] = attn_un[.., 0:64] / attn_un[.., 64] per head."""
    for sc in range(QSUB):
        rec = sb_small.tile([P, H], f32, tag="nrm_rec")
        nc.vector.reciprocal(rec, attn_un[:, sc, :, 64])
        nc.vector.tensor_tensor(
            attn8[:, sc].rearrange("p (h w) -> p h w", h=H),
            attn_un[:, sc, :, 0:64],
            rec[:, :, None].to_broadcast((P, H, 64)),
            ALU.mult)


def _out_proj(nc, tc, halves, aoutT8, bias_row, ones_row, resid):
    """resid = resid + aout @ Wo + bias (bias via rank-1 bf16 matmul)."""
    with (
        tc.tile_pool(name="wo_ps", bufs=2, space="PSUM") as ps_pool,
    ):
        for sc in range(QSUB):
            ps = ps_pool.tile([P, 2, 512], f32, tag="wo")
            for dh in range(2):
                nc.tensor.matmul(
                    ps[:, dh], ones_row, bias_row[:, dh * 512:(dh + 1) * 512],
                    start=True, stop=False, skip_group_check=True,
                )
                for q in range(KP):
                    nc.tensor.matmul(
                        ps[:, dh],
                        aoutT8[:, 2 * q:2 * q + 2, sc * P:(sc + 1) * P],
                        halves[q][:, :, dh * 512:(dh + 1) * 512],
                        start=False, stop=(q == KP - 1), perf_mode=DR,
                        skip_group_check=True,
                    )
            nc.vector.scalar_tensor_tensor(
                resid[:, sc], ps.rearrange("p a b -> p (a b)"),
                1.0 / (WS * WS), resid[:, sc], ALU.mult, ALU.add)


def _ln_transpose_q(nc, tc, sb_small, ident8, h_in, xnT8, eps_ap, eng,
                    xnT8_lo=None):
    """LN each of the 4 h-chunks and transpose into xnT8 [P, DSUB, SQ].

    If xnT8_lo is given, also write the fp8 quantization residual
    (bf16(xn) - fp8(xn)) for hi/lo double-fp8 matmuls.
    """
    with (
        tc.tile_pool(name="lnq", bufs=3) as xn_pool,
        tc.tile_pool(name="lnq_tr", bufs=3, space="PSUM") as ps_tr,
    ):
        for sc in range(QSUB):
            xn8 = xn_pool.tile([P, D], bf16, tag="xn")
            _ln_tile(nc, sb_small, h_in[:, sc], xn8, eps_ap)
            for a in range(2):
                tp = ps_tr.tile([P, 4, P], bf16, tag="tr_ps")
                for i in range(4):
                    nc.tensor.transpose(
                        tp[:, i],
                        xn8[:, a * 512 + i * P:a * 512 + (i + 1) * P], ident8)
                hi = xnT8[:, 4 * a:4 * a + 4, sc * P:(sc + 1) * P]
                if eng == "act":
                    nc.scalar.copy(hi, tp)
                else:
                    nc.vector.tensor_copy(hi, tp)
                if xnT8_lo is not None:
                    nc.vector.tensor_tensor(
                        xnT8_lo[:, 4 * a:4 * a + 4, sc * P:(sc + 1) * P],
                        tp, hi, ALU.subtract)


def _transpose_aout(nc, tc, ident8, attn8, aoutT8):
    with tc.tile_pool(name="aout_tr", bufs=3, space="PSUM") as ps_tr:
        for sc in range(QSUB):
            for a in range(2):
                _transpose4(
                    nc, ps_tr, ident8, attn8[:, sc, a * 512:(a + 1) * 512],
                    aoutT8[:, 4 * a:4 * a + 4, sc * P:(sc + 1) * P], "act")


# --------------------------------------------------------------------------
# full program
# --------------------------------------------------------------------------

def build_nc(reps=1):
    nc = bacc.Bacc(None, target_bir_lowering=False, debug=False)

    x_dr = nc.dram_tensor("x", [SQ, D], f32, kind="ExternalInput")
    xhi_dr = nc.dram_tensor("x_hi_bf", [SQ, D], bf16, kind="ExternalInput")
    ctxT_dr = nc.dram_tensor("ctxT8", [D, T], f8, kind="ExternalInput")
    maskd_dr = nc.dram_tensor("maskd", [T], f32, kind="ExternalInput")
    wdr = {}
    for a in (1, 2):
        for nm in ("Wq", "Wk", "Wv", "Wo"):
            wdr[f"{nm}{a}"] = nc.dram_tensor(
                f"{nm}{a}", [D, D], f8, kind="ExternalInput")
    qb1_dr = nc.dram_tensor("qb1", [D], f32, kind="ExternalInput")
    kb1_dr = nc.dram_tensor("kb1", [D], f32, kind="ExternalInput")
    vb1_dr = nc.dram_tensor("vb1", [D], f32, kind="ExternalInput")
    qb2_dr = nc.dram_tensor("qb2", [D], f32, kind="ExternalInput")
    bo1_dr = nc.dram_tensor("bo1x", [D], bf16, kind="ExternalInput")
    bo2_dr = nc.dram_tensor("bo2x", [D], bf16, kind="ExternalInput")
    bff2_dr = nc.dram_tensor("bff2x", [D], bf16, kind="ExternalInput")
    wff1_dr = nc.dram_tensor("Wff1hl", [2 * D, 2 * FF], f8,
                             kind="ExternalInput")
    bff1h_dr = nc.dram_tensor("bff1h8", [FF], f32, kind="ExternalInput")
    bff1g_dr = nc.dram_tensor("bff1g", [FF], f32, kind="ExternalInput")
    wff2_dr = nc.dram_tensor("Wff2hl", [2 * FF, D], f8,
                             kind="ExternalInput")
    out_dr = nc.dram_tensor("out", [SQ, D], f32, kind="ExternalOutput")

    x_tiled = x_dr.rearrange("(ss p) d -> p ss d", p=P)
    xhi_tiled = xhi_dr.rearrange("(ss p) d -> p ss d", p=P)

    with tile.TileContext(nc) as tc, contextlib.ExitStack() as es:
        const = es.enter_context(tc.tile_pool(name="const", bufs=1))
        sb_small = es.enter_context(tc.tile_pool(name="smalls", bufs=6))

        ident8 = const.tile([P, P], bf16)
        make_identity(nc, ident8)
        eps_ap = const.tile([P, 1], f32)
        nc.vector.memset(eps_ap, EPS)
        ones_row = const.tile([1, P], bf16)
        nc.vector.memset(ones_row, 1.0)
        bo1_row = const.tile([1, D], bf16)
        nc.sync.dma_start(bo1_row, bo1_dr[None, :])
        bo2_row = const.tile([1, D], bf16)
        nc.sync.dma_start(bo2_row, bo2_dr[None, :])
        bff2_row = const.tile([1, D], bf16)
        nc.sync.dma_start(bff2_row, bff2_dr[None, :])
        vb1_b = const.tile([P, D], f32)
        nc.sync.dma_start(vb1_b, vb1_dr[None, :].to_broadcast((P, D)))
        qb1_s = const.tile([P, DSUB], f32)
        nc.sync.dma_start(qb1_s, qb1_dr.rearrange("(c p) -> p c", p=P))
        kb1_s = const.tile([P, DSUB], f32)
        nc.sync.dma_start(kb1_s, kb1_dr.rearrange("(c p) -> p c", p=P))
        qb2_s = const.tile([P, DSUB], f32)
        nc.sync.dma_start(qb2_s, qb2_dr.rearrange("(c p) -> p c", p=P))
        bff1h_s = const.tile([P, FSUB], f32)
        nc.sync.dma_start(bff1h_s, bff1h_dr.rearrange("(c p) -> p c", p=P))
        bff1g_s = const.tile([P, FSUB], f32)
        nc.sync.dma_start(bff1g_s, bff1g_dr.rearrange("(c p) -> p c", p=P))
        maskd_s = const.tile([P, TSUB], f32)
        nc.sync.dma_start(maskd_s, maskd_dr.rearrange("(c p) -> p c", p=P))

        for _rep in range(reps):
            # Residual buffer, reused in place: x_q -> h1 -> h2 -> out.
            hbuf, free_hbuf = tc.tile([P, QSUB, D], f32, name="hbuf")
            with tc.high_priority():
                for sc in range(QSUB):
                    nc.sync.dma_start(hbuf[:, sc], x_tiled[:, sc])

            # Attention staging buffers shared by both blocks (freed after
            # wo2) — allocated first to keep pool lifetimes LIFO.
            attn_un, free_attn_un = tc.tile([P, QSUB, H, 65], f8,
                                            name="attn_un")
            attn8, free_attn8 = tc.tile([P, QSUB, D], bf16, name="attn8")

            # Cross-attention K2/V2 depend only on ctx — allocate their
            # tiles before block1 so the scheduler can hoist their
            # projections into attn1's ACT-bound window (no SBUF aliasing
            # anti-deps on block1 buffers).
            w2_es = contextlib.ExitStack()
            wpool2 = w2_es.enter_context(tc.tile_pool(name="w2", bufs=16))
            ps_kv2 = w2_es.enter_context(
                tc.tile_pool(name="kv2_ps", bufs=2, space="PSUM"))
            k2T8, free_k2T = tc.tile([P, 4, 2, T], f8, name="k2T8")
            v2, free_v2 = tc.tile([P, TSUB, H, 65], f8, name="v2")
            ctxT_sb, free_ctxT = tc.tile([P, DSUB, T], f8, name="ctxT_sb")
            _ctxT_t = ctxT_dr.rearrange("(ds p) t -> p ds t", p=P)
            w1_es = contextlib.ExitStack()
            wpool1 = w1_es.enter_context(tc.tile_pool(name="w1", bufs=16))

            # ---- Phase 1: LN1 + transpose (full S rows) ----
            xn1T, free_xn1T = tc.tile([P, DSUB, S], f8, name="xn1T")
            # x first: the upper-half tiles feed LN1 immediately; everything
            # below must not queue ahead of them on the DMA engines
            x_hi4, free_x_hi4 = tc.tile([P, 4, D], bf16, name="x_hi4")
            with tc.high_priority():
                for ss in range(QSUB, SSUB):
                    nc.sync.dma_start(x_hi4[:, ss - QSUB],
                                      xhi_tiled[:, ss - QSUB])
            # prefetch every attention weight now: dedicated buffers, early
            # DMA-queue slots (weight loads must never sit on the critical
            # path mid-kernel)
            for ds in range(DSUB):
                nc.sync.dma_start(ctxT_sb[:, ds], _ctxT_t[:, ds])
            w_q1 = _load_w8(nc, wpool1, wdr["Wq1"])
            w_k1 = _load_w8(nc, wpool1, wdr["Wk1"])
            w_v1 = _load_w8(nc, wpool1, wdr["Wv1"])
            w_o1 = _load_w8(nc, wpool1, wdr["Wo1"])
            w_k2 = _load_w8(nc, wpool2, wdr["Wk2"])
            w_v2 = _load_w8(nc, wpool2, wdr["Wv2"])
            w_q2 = _load_w8(nc, wpool2, wdr["Wq2"])
            w_o2 = _load_w8(nc, wpool2, wdr["Wo2"])
            if _KSTOP >= 1:
             with (
                nc.named_scope("ln1"),
                tc.tile_pool(name="xn1", bufs=3) as xn1_pool,
                tc.tile_pool(name="tr1_ps", bufs=3, space="PSUM") as ps_tr,
            ):
                for ss in range(SSUB):
                    if ss < QSUB:
                        xt = hbuf[:, ss]
                    else:
                        xt = x_hi4[:, ss - QSUB]
                    xn8 = xn1_pool.tile([P, D], bf16, tag="xn1")
                    _ln_tile(nc, sb_small, xt, xn8, eps_ap)
                    for a in range(2):
                        _transpose4(
                            nc, ps_tr, ident8, xn8[:, a * 512:(a + 1) * 512],
                            xn1T[:, 4 * a:4 * a + 4, ss * P:(ss + 1) * P],
                            "act")

            free_x_hi4()

            # ---- attention block 1 (self) ----
            q1T8, free_q1T = tc.tile([P, 4, 2, SQ], f8, name="q1T8")
            k1T8, free_k1T = tc.tile([P, 4, 2, T], f8, name="k1T8")
            v1, free_v1 = tc.tile([P, TSUB, H, 65], f8, name="v1")
            if _KSTOP >= 2:
             with (
                nc.named_scope("qkv1"),
                tc.tile_pool(name="qkv1_ps", bufs=6, space="PSUM") as ps_proj,
            ):
                qs = list(_proj_qk_steps(nc, w_q1, ps_proj, xn1T[:, :, :SQ],
                                          q1T8, qb1_s, SQ, "dve"))
                ks = list(_proj_qk_steps(nc, w_k1, ps_proj, xn1T,
                                         k1T8, kb1_s, T, "act"))
                vs = list(_proj_v_steps(nc, w_v1, ps_proj, xn1T, v1,
                                        vb1_b, None))
                # head-group-major; K ch0 units first (they only need the
                # query-half transposes), ch1 units after
                for b in range(4):
                    for u in (qs[2 * b], qs[2 * b + 1], ks[4 * b],
                              ks[4 * b + 2]):
                        u()
                for b in range(4):
                    ks[4 * b + 1]()
                    ks[4 * b + 3]()
                for ts in range(TSUB):
                    vs[2 * ts]()
                for ts in range(TSUB):
                    vs[2 * ts + 1]()
                vs[16]()
            kv2_filler = None
            if _KSTOP >= 6:
                import itertools
                kv2_filler = itertools.chain(
                    _proj_qk_steps(nc, w_k2, ps_kv2, ctxT_sb, k2T8, None, T,
                                   "dve"),
                    _proj_v_steps(nc, w_v2, ps_kv2, ctxT_sb, v2, None,
                                  maskd_s))
            if _KSTOP >= 3:
             with (
                nc.named_scope("attn1"),
                tc.tile_pool(name="ET1", bufs=2) as ET_pool,
                tc.tile_pool(name="sc1_ps", bufs=2, space="PSUM") as ps_sc,
                tc.tile_pool(name="pv1_ps", bufs=2, space="PSUM") as ps_pv,
            ):
                _attention(nc, ET_pool, ps_sc, ps_pv, sb_small, q1T8, k1T8,
                           v1, attn_un, filler=kv2_filler)
            free_v1(); free_k1T(); free_q1T()
            aout1T, free_aout1T = tc.tile([P, DSUB, SQ], f8, name="aout1T")
            if _KSTOP >= 4:
             with nc.named_scope("wo1"):
                _normalize(nc, sb_small, attn_un, attn8)
                _transpose_aout(nc, tc, ident8, attn8, aout1T)
                _out_proj(nc, tc, w_o1, aout1T, bo1_row, ones_row, hbuf)
            free_aout1T(); free_xn1T(); w1_es.close()

            # ---- attention block 2 (cross) ----
            xn2T, free_xn2T = tc.tile([P, DSUB, SQ], f8, name="xn2T")
            q2T8, free_q2T = tc.tile([P, 4, 2, SQ], f8, name="q2T8")
            if _KSTOP >= 5:
             with nc.named_scope("ln2"):
                _ln_transpose_q(nc, tc, sb_small, ident8, hbuf, xn2T, eps_ap,
                                "act")
            if _KSTOP >= 6:
             with nc.named_scope("qkv2"):
                if kv2_filler is not None:
                    for f in kv2_filler:
                        f()
                _proj_qk(nc, w_q2, ps_kv2, xn2T,
                         q2T8, qb2_s, SQ, "act")
            if _KSTOP >= 7:
             with (
                nc.named_scope("attn2"),
                tc.tile_pool(name="ET2", bufs=2) as ET_pool,
                tc.tile_pool(name="sc2_ps", bufs=2, space="PSUM") as ps_sc,
                tc.tile_pool(name="pv2_ps", bufs=2, space="PSUM") as ps_pv,
            ):
                _attention(nc, ET_pool, ps_sc, ps_pv, sb_small, q2T8, k2T8,
                           v2, attn_un)
            free_q2T()
            aout2T, free_aout2T = tc.tile([P, DSUB, SQ], f8, name="aout2T")
            if _KSTOP >= 8:
             with nc.named_scope("wo2"):
                _normalize(nc, sb_small, attn_un, attn8)
                _transpose_aout(nc, tc, ident8, attn8, aout2T)
                _out_proj(nc, tc, w_o2, aout2T, bo2_row, ones_row, hbuf)
            free_aout2T(); free_xn2T()
            free_ctxT(); free_v2(); free_k2T(); w2_es.close()
            free_attn8(); free_attn_un()

            # ---- GEGLU feed-forward ----
            xn3T, free_xn3T = tc.tile([P, 2, DSUB, SQ], f8, name="xn3T")
            if _KSTOP >= 9:
             with nc.named_scope("ln3"):
                _ln_transpose_q(nc, tc, sb_small, ident8, hbuf, xn3T[:, 0],
                                eps_ap, "act", xnT8_lo=xn3T[:, 1])

            mT8, free_mT8 = tc.tile([P, FSUB, SQ], f8, name="mT8")
            ff_es = contextlib.ExitStack()
            wff2_pool = ff_es.enter_context(tc.tile_pool(name="wff2", bufs=6))
            wff1_t = wff1_dr.rearrange("(ks p) f -> p ks f", p=P)
            if _KSTOP >= 10:
             with (
                nc.named_scope("ff1"),
                tc.tile_pool(name="wff1", bufs=4) as wff1_pool,
                tc.tile_pool(name="ff1_ps", bufs=6, space="PSUM") as ps_ff1,
                tc.tile_pool(name="hT", bufs=3) as hT_pool,
                tc.tile_pool(name="gT", bufs=3) as gT_pool,
            ):
                for grp in range(8):
                    wh = wff1_pool.tile([P, 2 * DSUB, 512], f8, tag="wff1h")
                    nc.sync.dma_start(
                        wh, wff1_t[:, :, grp * 512:(grp + 1) * 512])
                    wg = wff1_pool.tile([P, 2 * DSUB, 512], f8, tag="wff1g")
                    nc.sync.dma_start(
                        wg, wff1_t[:, :, FF + grp * 512:FF + (grp + 1) * 512])
                    for fi in range(4):
                        fc = grp * 4 + fi
                        hg = []
                        for part_i, wt in ((0, wh), (1, wg)):
                            ps = ps_ff1.tile([P, SQ], f32, tag="yT")
                            if part_i == 0:
                                # h-part: (A_q, B_q) x dup(x_hi_q) — exact W
                                for q in range(DSUB):
                                    nc.tensor.matmul(
                                        ps,
                                        wt[:, 2 * q:2 * q + 2,
                                           fi * P:(fi + 1) * P],
                                        xn3T[:, 0, q:q + 1, :].to_broadcast(
                                            (P, 2, SQ)),
                                        start=(q == 0), stop=False,
                                        perf_mode=DR,
                                        skip_group_check=(q > 0))
                                # + (A_2q, A_2q+1) x (x_lo pair)
                                for q2 in range(KP):
                                    nc.tensor.matmul(
                                        ps,
                                        wt[:, 4 * q2:4 * q2 + 4:2,
                                           fi * P:(fi + 1) * P],
                                        xn3T[:, 1, 2 * q2:2 * q2 + 2, :],
                                        start=False, stop=(q2 == KP - 1),
                                        perf_mode=DR, skip_group_check=True)
                            else:
                                # gate: plain fp8 (A slots, x_hi pairs) —
                                # gelu' squashes the quantization error
                                for q2 in range(KP):
                                    nc.tensor.matmul(
                                        ps,
                                        wt[:, 4 * q2:4 * q2 + 4:2,
                                           fi * P:(fi + 1) * P],
                                        xn3T[:, 0, 2 * q2:2 * q2 + 2, :],
                                        start=(q2 == 0), stop=(q2 == KP - 1),
                                        perf_mode=DR,
                                        skip_group_check=(q2 > 0))
                            if part_i == 0:
                                hT = hT_pool.tile([P, SQ], bf16, tag="hT")
                                nc.vector.tensor_scalar(
                                    hT, ps, MS / WS, bff1h_s[:, fc:fc + 1],
                                    ALU.mult, ALU.add)
                                hg.append(hT)
                            else:
                                gT = gT_pool.tile([P, SQ], bf16, tag="gT")
                                nc.scalar.activation(
                                    gT, ps, AF.Gelu,
                                    bias=bff1g_s[:, fc:fc + 1], scale=1.0 / WS)
                                hg.append(gT)
                        nc.gpsimd.tensor_tensor(
                            mT8[:, fc, :], hg[0], hg[1], ALU.mult)

            wff2_t = wff2_dr.rearrange("(ks p) o -> p ks o", p=P)
            if _KSTOP >= 11:
             with (
                nc.named_scope("ff2"),
                tc.tile_pool(name="ff2_ps", bufs=1, space="PSUM") as ps_ff2,
            ):
                ps_o = [ps_ff2.tile([P, 512], f32, tag=f"o{i}", name=f"ps_o{i}")
                        for i in range(8)]
                for sc in range(QSUB):
                    for dh in range(2):
                        nc.tensor.matmul(
                            ps_o[sc * 2 + dh], ones_row,
                            bff2_row[:, dh * 512:(dh + 1) * 512],
                            start=True, stop=False, skip_group_check=True)
                for k in range(FSUB):
                    wt = wff2_pool.tile([P, 2, D], f8, tag="wff2")
                    nc.sync.dma_start(wt, wff2_t[:, 2 * k:2 * k + 2])
                    for sc in range(QSUB):
                        for dh in range(2):
                            # dup(m_k) x (A2_k, B2_k) — exact Wff2
                            nc.tensor.matmul(
                                ps_o[sc * 2 + dh],
                                mT8[:, k:k + 1,
                                    sc * P:(sc + 1) * P].to_broadcast(
                                        (P, 2, P)),
                                wt[:, :, dh * 512:(dh + 1) * 512],
                                start=False, stop=(k == FSUB - 1),
                                perf_mode=DR, skip_group_check=True)
                for sc in range(QSUB):
                    for dh in range(2):
                        sl = slice(dh * 512, (dh + 1) * 512)
                        nc.vector.scalar_tensor_tensor(
                            hbuf[:, sc, sl], ps_o[sc * 2 + dh], 1.0 / (MS * WS),
                            hbuf[:, sc, sl], ALU.mult, ALU.add)
            ff_es.close(); free_mT8(); free_xn3T()

            out_t = out_dr.rearrange("(ss p) d -> p ss d", p=P)
            for sc in range(QSUB):
                nc.sync.dma_start(out_t[:, sc], hbuf[:, sc])
            free_hbuf()

    nc.compile()
    return nc


# --------------------------------------------------------------------------
# host side
# --------------------------------------------------------------------------

_NC = None


def _get_nc():
    global _NC
    if _NC is None:
        _NC = build_nc()
    return _NC


def _f8(a):
    import ml_dtypes
    return np.ascontiguousarray(
        np.clip(np.asarray(a, np.float32), -240.0, 240.0).astype(
            ml_dtypes.float8_e4m3))


def _f8_hilo(Ws):
    """[Din, O] f32 (pre-scaled) -> [2*Din, O] f8, per-128-row-block
    interleaved (A_0, B_0, A_1, B_1, ...) where A = f8(W), B = f8(W - A)."""
    Ws = np.asarray(Ws, np.float32)
    A = _f8(Ws)
    Bq = _f8(Ws - A.astype(np.float32))
    Din, O = Ws.shape
    out = np.empty((2 * Din, O), dtype=A.dtype)
    nb = Din // P
    for q in range(nb):
        out[(2 * q) * P:(2 * q + 1) * P] = A[q * P:(q + 1) * P]
        out[(2 * q + 1) * P:(2 * q + 2) * P] = Bq[q * P:(q + 1) * P]
    return np.ascontiguousarray(out)


def _scores_perm():
    """Column permutation for Q/K weights: new col j <- old col d(j)."""
    j = np.arange(D)
    b, r = j // 256, j % 256
    i, p = r // 128, r % 128
    h4, dv32 = p // 32, p % 32
    return (4 * b + h4) * 64 + i * 32 + dv32


def _make_in_maps(inputs):
    f = np.float32
    hidden = np.asarray(inputs["hidden_states"], f)
    context = np.asarray(inputs["context"], f)
    mask = np.asarray(inputs["encoder_key_padding_mask"]).astype(f)
    g1, b1 = np.asarray(inputs["g1"], f), np.asarray(inputs["b1"], f)
    g2, b2 = np.asarray(inputs["g2"], f), np.asarray(inputs["b2"], f)
    g3, b3 = np.asarray(inputs["g3"], f), np.asarray(inputs["b3"], f)

    def fold(g, W):
        return g[:, None] * np.asarray(W, f)

    perm = _scores_perm()

    Wq1 = fold(g1, inputs["Wq1"])[:, perm]
    Wk1 = fold(g1, inputs["Wk1"])[:, perm]
    Wv1 = fold(g1, inputs["Wv1"])
    Wo1 = np.asarray(inputs["Wo1"], f)
    qb1 = (b1 @ np.asarray(inputs["Wq1"], f))[perm]
    kb1 = (b1 @ np.asarray(inputs["Wk1"], f))[perm]
    vb1 = b1 @ np.asarray(inputs["Wv1"], f)
    Wq2 = fold(g2, inputs["Wq2"])[:, perm]
    Wk2 = np.asarray(inputs["Wk2"], f)[:, perm]
    Wv2 = np.asarray(inputs["Wv2"], f)
    Wo2 = np.asarray(inputs["Wo2"], f)
    qb2 = (b2 @ np.asarray(inputs["Wq2"], f))[perm]
    Wff1 = fold(g3, inputs["Wff1"])
    bff1 = np.asarray(inputs["bff1"], f) + b3 @ np.asarray(inputs["Wff1"], f)
    Wff2 = np.asarray(inputs["Wff2"], f)

    import ml_dtypes
    bfa = lambda a: np.ascontiguousarray(np.asarray(a, f).astype(
        ml_dtypes.bfloat16))

    shared = {
        "Wq1": _f8(Wq1 * WS), "Wk1": _f8(Wk1 * WS), "Wv1": _f8(Wv1 * WS),
        "Wo1": _f8(Wo1 * WS),
        "qb1": np.ascontiguousarray(qb1), "kb1": np.ascontiguousarray(kb1),
        "vb1": np.ascontiguousarray(vb1),
        "bo1x": bfa(np.asarray(inputs["bo1"], f) * WS * WS),
        "Wq2": _f8(Wq2 * WS), "Wk2": _f8(Wk2 * WS), "Wv2": _f8(Wv2 * WS),
        "Wo2": _f8(Wo2 * WS),
        "qb2": np.ascontiguousarray(qb2),
        "bo2x": bfa(np.asarray(inputs["bo2"], f) * WS * WS),
        "Wff1hl": _f8_hilo(Wff1 * WS),
        "bff1h8": np.ascontiguousarray(bff1[:FF] * MS),
        "bff1g": np.ascontiguousarray(bff1[FF:]),
        "Wff2hl": _f8_hilo(Wff2 * WS),
        "bff2x": bfa(np.asarray(inputs["bff2"], f) * MS * WS),
    }

    import ml_dtypes
    in_maps = []
    for core in range(NCORES):
        b, q = core // 2, core % 2
        x = hidden[b] if q == 0 else np.roll(hidden[b], -SQ, axis=0)
        in_maps.append({
            **shared,
            "x": np.ascontiguousarray(x[:SQ]),
            "x_hi_bf": np.ascontiguousarray(
                x[SQ:].astype(ml_dtypes.bfloat16)),
            "ctxT8": _f8(context[b].T),
            "maskd": np.ascontiguousarray(mask[b] / WS),
        })
    return in_maps


def run(inputs, **spmd_kwargs):
    res = run_bass_kernel_spmd(
        _get_nc(), _make_in_maps(inputs), core_ids=list(range(NCORES)),
        **spmd_kwargs)
    out = np.empty((B, S, D), np.float32)
    for core in range(NCORES):
        b, q = core // 2, core % 2
        out[b, q * SQ:(q + 1) * SQ] = res.results[core]["out"]
    return out, res


def kernel(**inputs):
    out, _ = run(inputs)
    return out
